# revision 1
# baseline (speedup 1.0000x reference)
"""Trainium2 Bass kernel for a dense-MoE FFN layer (top-2 routing).

Expert-parallel over 8 NeuronCores: core e owns expert e (W1[e], W2[e]).
Every core:
  - computes fp32 router logits for all tokens (replicated router),
    derives its own expert's per-token top-2 softmax weight on device,
  - runs the dense expert FFN in bf16 (fp32 accumulate in PSUM),
  - scales by the router weight, writes a partial sum [N, D],
  - ReduceScatter(+) over the 8 cores -> each core holds the summed
    MoE output for a distinct 512-token slice,
  - adds the residual and applies LayerNorm on that slice.
The host concatenates the 8 slices into the full [B, S, D] output.
"""

import numpy as np
import ml_dtypes

B, S, D, F, E = 2, 2048, 1024, 4096, 8
N = B * S              # 4096 tokens
NC = 8                 # cores
TSLICE = N // NC       # 512 tokens output slice per core
TB = 512               # token block for the matmul pipeline
NB = N // TB           # 8 blocks
ND = D // 128          # 8 d-tiles
NF = F // 128          # 32 f-tiles
NT = N // 128          # 32 token tiles
LN_EPS = 1e-5

BF16 = ml_dtypes.bfloat16

_CACHE = {}


def _build_nc(do_collective=True, n_blocks=NB, do_router=True):
    import concourse.bacc as bacc
    import concourse.mybir as mybir
    import concourse.tile as tile

    dt = mybir.dt
    f32, bf16 = dt.float32, dt.bfloat16
    Alu = mybir.AluOpType
    Act = mybir.ActivationFunctionType
    AX = mybir.AxisListType.X

    nc = bacc.Bacc(num_devices=NC)

    xtf = nc.dram_tensor("xtf", [D, N], f32, kind="ExternalInput")
    xtb = nc.dram_tensor("xtb", [D, N], bf16, kind="ExternalInput")
    w1t = nc.dram_tensor("w1t", [D, F], bf16, kind="ExternalInput")
    w2t = nc.dram_tensor("w2t", [F, D], bf16, kind="ExternalInput")
    b1c = nc.dram_tensor("b1c", [128, NF], f32, kind="ExternalInput")
    b2r = nc.dram_tensor("b2r", [128, D], f32, kind="ExternalInput")
    wrt = nc.dram_tensor("wrt", [D, E], f32, kind="ExternalInput")
    brr = nc.dram_tensor("brr", [128, E], f32, kind="ExternalInput")
    xres = nc.dram_tensor("xres", [TSLICE, D], f32, kind="ExternalInput")
    gmr = nc.dram_tensor("gmr", [128, D], f32, kind="ExternalInput")
    btr = nc.dram_tensor("btr", [128, D], f32, kind="ExternalInput")
    out = nc.dram_tensor("out", [TSLICE, D], f32, kind="ExternalOutput")

    xtf_r = xtf.ap().rearrange("(a p) n -> a p n", p=128)
    xtb_r = xtb.ap().rearrange("(a p) n -> a p n", p=128)
    w1t_r = w1t.ap().rearrange("(a p) f -> a p f", p=128)
    w2t_r = w2t.ap().rearrange("(a p) d -> a p d", p=128)
    wrt_r = wrt.ap().rearrange("(a p) e -> a p e", p=128)
    xres_r = xres.ap().rearrange("(a p) d -> a p d", p=128)
    out_r = out.ap().rearrange("(a p) d -> a p d", p=128)

    with tile.TileContext(nc) as tc:
        with (
            tc.tile_pool(name="wts", bufs=1) as wts,
            tc.tile_pool(name="xs", bufs=1) as xs_pool,
            tc.tile_pool(name="stage", bufs=4) as stage_pool,
            tc.tile_pool(name="psr", bufs=2, space="PSUM") as psum_r,
            tc.tile_pool(name="psh", bufs=2, space="PSUM") as psum_h,
            tc.tile_pool(name="pso", bufs=4, space="PSUM") as psum_o,
            tc.tile_pool(name="dram", bufs=1, space="DRAM") as dram,
        ):
            # --- persistent small tensors ---
            wrt_sb = []
            for d0 in range(ND):
                t = wts.tile([128, E], f32, name=f"wrt{d0}", tag=f"wrt{d0}")
                nc.sync.dma_start(t[:], wrt_r[d0])
                wrt_sb.append(t)
            brr_sb = wts.tile([128, E], f32, name="brr_sb")
            nc.sync.dma_start(brr_sb[:], brr[:])
            b1_sb = wts.tile([128, NF], f32, name="b1_sb")
            nc.sync.dma_start(b1_sb[:], b1c[:])
            b2_sb = wts.tile([128, D], f32, name="b2_sb")
            nc.sync.dma_start(b2_sb[:], b2r[:])
            gm_sb = wts.tile([128, D], f32, name="gm_sb")
            nc.sync.dma_start(gm_sb[:], gmr[:])
            bt_sb = wts.tile([128, D], f32, name="bt_sb")
            nc.sync.dma_start(bt_sb[:], btr[:])
            # per-token router weight for this core's expert, [128, NT]
            w_all = wts.tile([128, NT], f32, name="w_all")
            eps_sb = wts.tile([128, 1], f32, name="eps_sb")
            nc.vector.memset(eps_sb[:], LN_EPS)
            if not do_router:
                nc.vector.memset(w_all[:], 0.5)

            # --- expert weights (persistent, stream in behind the router) ---
            w1_sb = []
            for d0 in range(ND):
                t = wts.tile([128, F], bf16, name=f"w1_{d0}", tag=f"w1_{d0}")
                nc.sync.dma_start(t[:], w1t_r[d0])
                w1_sb.append(t)
            w2_sb = []
            for f0 in range(NF):
                t = wts.tile([128, D], bf16, name=f"w2_{f0}", tag=f"w2_{f0}")
                nc.sync.dma_start(t[:], w2t_r[f0])
                w2_sb.append(t)

            # --- router phase: fp32 logits -> top-2 weight for own expert ---
            with (
                tc.tile_pool(name="xtfp", bufs=2) as xtf_pool,
                tc.tile_pool(name="rtmp", bufs=4) as rtmp,
            ):
                for blk in range(NB if do_router else 0):
                    xf = []
                    for d0 in range(ND):
                        t = xtf_pool.tile([128, TB], f32, name=f"xf{d0}", tag=f"xf{d0}")
                        nc.sync.dma_start(t[:], xtf_r[d0][:, blk * TB:(blk + 1) * TB])
                        xf.append(t)
                    for tt in range(TB // 128):
                        tok = blk * (TB // 128) + tt
                        ps = psum_r.tile([128, E], f32, name="ps_r", tag="ps_r")
                        for d0 in range(ND):
                            nc.tensor.matmul(
                                ps[:],
                                lhsT=xf[d0][:, tt * 128:(tt + 1) * 128],
                                rhs=wrt_sb[d0][:],
                                start=(d0 == 0),
                                stop=(d0 == ND - 1),
                            )
                        lg = rtmp.tile([128, E], f32, name="lg", tag="lg")
                        nc.vector.tensor_tensor(lg[:], ps[:], brr_sb[:], op=Alu.add)
                        m1 = rtmp.tile([128, 1], f32, name="m1", tag="m1")
                        nc.vector.reduce_max(m1[:], lg[:], axis=AX)
                        eq = rtmp.tile([128, E], f32, name="eq", tag="eq")
                        nc.vector.tensor_scalar(
                            eq[:], lg[:], m1[:], None, op0=Alu.is_equal
                        )
                        msk = rtmp.tile([128, E], f32, name="msk", tag="msk")
                        nc.vector.scalar_tensor_tensor(
                            msk[:], in0=eq[:], scalar=-1e30, in1=lg[:],
                            op0=Alu.mult, op1=Alu.add,
                        )
                        m2 = rtmp.tile([128, 1], f32, name="m2", tag="m2")
                        nc.vector.reduce_max(m2[:], msk[:], axis=AX)
                        my = lg[:, 0:1]
                        d1 = rtmp.tile([128, 1], f32, name="d1", tag="d1")
                        nc.vector.tensor_tensor(d1[:], my, m2[:], op=Alu.subtract)
                        d2 = rtmp.tile([128, 1], f32, name="d2", tag="d2")
                        nc.vector.tensor_tensor(d2[:], my, m1[:], op=Alu.subtract)
                        s1 = rtmp.tile([128, 1], f32, name="s1", tag="s1")
                        nc.scalar.activation(s1[:], d1[:], Act.Sigmoid)
                        s2 = rtmp.tile([128, 1], f32, name="s2", tag="s2")
                        nc.scalar.activation(s2[:], d2[:], Act.Sigmoid)
                        e1 = rtmp.tile([128, 1], f32, name="e1", tag="e1")
                        nc.vector.tensor_tensor(e1[:], my, m1[:], op=Alu.is_equal)
                        e2 = rtmp.tile([128, 1], f32, name="e2", tag="e2")
                        nc.vector.tensor_tensor(e2[:], my, m2[:], op=Alu.is_equal)
                        t1 = rtmp.tile([128, 1], f32, name="t1", tag="t1")
                        nc.vector.tensor_tensor(t1[:], e1[:], s1[:], op=Alu.mult)
                        t2 = rtmp.tile([128, 1], f32, name="t2", tag="t2")
                        nc.vector.tensor_tensor(t2[:], e2[:], s2[:], op=Alu.mult)
                        nc.vector.tensor_tensor(
                            w_all[:, tok:tok + 1], t1[:], t2[:], op=Alu.add
                        )

            partial = dram.tile([N, D], f32, name="partial")
            partial_r = partial.rearrange("(t p) d -> t p d", p=128)

            # --- expert FFN blocks ---
            with tc.tile_pool(name="htp", bufs=1) as ht_pool:
                for blk in range(n_blocks):
                    xb = []
                    for d0 in range(ND):
                        t = xs_pool.tile([128, TB], bf16, name=f"xb{d0}", tag=f"xb{d0}")
                        nc.sync.dma_start(t[:], xtb_r[d0][:, blk * TB:(blk + 1) * TB])
                        xb.append(t)
                    # h^T = relu(W1 x^T + b1), produced as 32 [128f, TB] bf16 tiles
                    ht = []
                    for f0 in range(NF):
                        hp = psum_h.tile([128, TB], f32, name="hp", tag="hp")
                        for d0 in range(ND):
                            nc.tensor.matmul(
                                hp[:],
                                lhsT=w1_sb[d0][:, f0 * 128:(f0 + 1) * 128],
                                rhs=xb[d0][:],
                                start=(d0 == 0),
                                stop=(d0 == ND - 1),
                            )
                        hs = ht_pool.tile([128, TB], bf16, name=f"ht{f0}", tag=f"ht{f0}")
                        nc.scalar.activation(
                            hs[:], hp[:], Act.Relu, bias=b1_sb[:, f0:f0 + 1]
                        )
                        ht.append(hs)
                    # o = h W2^T + b2, weighted by router w, to partial DRAM
                    for ts in range(TB // 128):
                        tok = blk * (TB // 128) + ts
                        op0 = psum_o.tile([128, 512], f32, name="op0", tag="op")
                        op1 = psum_o.tile([128, 512], f32, name="op1", tag="op")
                        for f0 in range(NF):
                            nc.tensor.matmul(
                                op0[:],
                                lhsT=ht[f0][:, ts * 128:(ts + 1) * 128],
                                rhs=w2_sb[f0][:, 0:512],
                                start=(f0 == 0),
                                stop=(f0 == NF - 1),
                            )
                            nc.tensor.matmul(
                                op1[:],
                                lhsT=ht[f0][:, ts * 128:(ts + 1) * 128],
                                rhs=w2_sb[f0][:, 512:1024],
                                start=(f0 == 0),
                                stop=(f0 == NF - 1),
                            )
                        for dn, op_ in enumerate((op0, op1)):
                            st = stage_pool.tile([128, 512], f32, name="st", tag="st")
                            nc.vector.tensor_tensor(
                                st[:], op_[:], b2_sb[:, dn * 512:(dn + 1) * 512],
                                op=Alu.add,
                            )
                            nc.vector.tensor_scalar_mul(
                                st[:], st[:], w_all[:, tok:tok + 1]
                            )
                            nc.sync.dma_start(
                                partial_r[tok][:, dn * 512:(dn + 1) * 512], st[:]
                            )

                rs_out = dram.tile([TSLICE, D], f32, name="rs_out")
                if do_collective:
                    nc.gpsimd.collective_compute(
                        "ReduceScatter",
                        Alu.add,
                        replica_groups=[list(range(NC))],
                        ins=[partial.opt()],
                        outs=[rs_out.opt()],
                    )
            rs_r = rs_out.rearrange("(t p) d -> t p d", p=128)

            # --- residual + LayerNorm on own 512-token slice ---
            with tc.tile_pool(name="ln", bufs=1) as ln_pool:
                for i in range(TSLICE // 128):
                    rs_sb = ln_pool.tile([128, D], f32, name="rs_sb", tag="rs")
                    nc.sync.dma_start(rs_sb[:], rs_r[i])
                    xr = ln_pool.tile([128, D], f32, name="xr", tag="xr")
                    nc.sync.dma_start(xr[:], xres_r[i])
                    nc.vector.tensor_tensor(rs_sb[:], rs_sb[:], xr[:], op=Alu.add)
                    mu = ln_pool.tile([128, 1], f32, name="mu", tag="mu")
                    nc.vector.reduce_sum(mu[:], rs_sb[:], axis=AX)
                    nc.vector.tensor_scalar_mul(mu[:], mu[:], 1.0 / D)
                    xc = ln_pool.tile([128, D], f32, name="xc", tag="xc")
                    nc.vector.tensor_scalar_sub(xc[:], rs_sb[:], mu[:])
                    sq = ln_pool.tile([128, D], f32, name="sq", tag="sq")
                    var = ln_pool.tile([128, 1], f32, name="var", tag="var")
                    nc.scalar.activation(sq[:], xc[:], Act.Square, accum_out=var[:])
                    std = ln_pool.tile([128, 1], f32, name="std", tag="std")
                    nc.scalar.activation(
                        std[:], var[:], Act.Sqrt, scale=1.0 / D, bias=eps_sb[:]
                    )
                    rstd = ln_pool.tile([128, 1], f32, name="rstd", tag="rstd")
                    nc.vector.reciprocal(rstd[:], std[:])
                    o_sb = ln_pool.tile([128, D], f32, name="o_sb", tag="o")
                    nc.vector.scalar_tensor_tensor(
                        o_sb[:], in0=xc[:], scalar=rstd[:], in1=gm_sb[:],
                        op0=Alu.mult, op1=Alu.mult,
                    )
                    nc.vector.tensor_tensor(o_sb[:], o_sb[:], bt_sb[:], op=Alu.add)
                    nc.sync.dma_start(out_r[i], o_sb[:])

    nc.finalize()
    return nc


CCAP = 1280              # per-expert token capacity (mean load is ~1024)
NTG = CCAP // 128        # 10 gathered token tiles
GBLOCKS = [512, 512, 256]
PAD_IDX = 2 ** 30        # sentinel row index; skipped via bounds_check


def _build_nc_sparse(do_scatter=True, do_collective=True, do_memset=True, debug_partial=False, unmerged=False):
    """Top-2-sparse expert FFN: each core computes only the tokens routed to
    its expert (gathered+padded to CCAP on host), scales by host-computed
    router weights, scatters rows into a zeroed dense [N, D] bf16 partial via
    indirect DMA, then ReduceScatter + residual + LayerNorm.

    All large inputs are host-pre-swizzled into SBUF layout so each loads
    with a single full-bandwidth DMA:
      xgsw  [128, ND*CCAP]  x^T gathered, per-block-major: blk | d0 | t
      w1sw  [128, ND*F]     W1[e]^T tiles column-concatenated by d0
      w2sw  [128, NF*D]     W2[e]^T tiles column-concatenated by f0
      cst1  [128, D+NF+NTG] b2 replicated | b1 cols | router-w cols
      cst2  [128, 2*D]      gamma replicated | beta replicated
    """
    import concourse.bacc as bacc
    import concourse.mybir as mybir
    import concourse.tile as tile
    from concourse.bass import IndirectOffsetOnAxis

    dt = mybir.dt
    f32, bf16, i32 = dt.float32, dt.bfloat16, dt.int32
    Alu = mybir.AluOpType
    Act = mybir.ActivationFunctionType
    AX = mybir.AxisListType.X

    nc = bacc.Bacc(num_devices=NC)

    xgsw = nc.dram_tensor("xgsw", [128, ND * CCAP], bf16, kind="ExternalInput")
    w1sw = nc.dram_tensor("w1sw", [128, ND * F], bf16, kind="ExternalInput")
    w2sw = nc.dram_tensor("w2sw", [128, NF * D], bf16, kind="ExternalInput")
    cst1 = nc.dram_tensor("cst1", [128, D + NF + NTG], f32, kind="ExternalInput")
    cst2 = nc.dram_tensor("cst2", [128, 2 * D], f32, kind="ExternalInput")
    scat = nc.dram_tensor("scat", [128, NTG], i32, kind="ExternalInput")
    xres = nc.dram_tensor("xres", [TSLICE, D], f32, kind="ExternalInput")
    out = nc.dram_tensor("out", [TSLICE, D], f32, kind="ExternalOutput")

    xres_r = xres.ap().rearrange("(a p) d -> a p d", p=128)
    out_r = out.ap().rearrange("(a p) d -> a p d", p=128)

    # per-block column offsets into xgsw: block b occupies ND*tb columns
    blk_off = [0]
    for tb in GBLOCKS:
        blk_off.append(blk_off[-1] + ND * tb)

    with tile.TileContext(nc) as tc:
        with (
            tc.tile_pool(name="wts", bufs=1) as wts,
            tc.tile_pool(name="xs", bufs=1) as xs_pool,
            tc.tile_pool(name="stg", bufs=2) as stg_pool,
            tc.tile_pool(name="stb", bufs=3) as stb_pool,
            tc.tile_pool(name="psh", bufs=2, space="PSUM") as psum_h,
            tc.tile_pool(name="pso", bufs=4, space="PSUM") as psum_o,
            tc.tile_pool(name="dram", bufs=1, space="DRAM") as dram,
        ):
            # first gathered block prefetch wins the DMA queue ahead of weights
            xb0 = xs_pool.tile([128, ND * 512], bf16, name="xb0", tag="xb")
            if unmerged:
                for d0 in range(ND):
                    nc.sync.dma_start(
                        xb0[:, d0 * GBLOCKS[0]:(d0 + 1) * GBLOCKS[0]],
                        xgsw[:, d0 * GBLOCKS[0]:(d0 + 1) * GBLOCKS[0]])
            else:
                nc.sync.dma_start(xb0[:, :ND * GBLOCKS[0]],
                                  xgsw[:, blk_off[0]:blk_off[1]])
            w1_all = wts.tile([128, ND * F], bf16, name="w1_all")
            if unmerged:
                for d0 in range(ND):
                    nc.sync.dma_start(w1_all[:, d0 * F:(d0 + 1) * F],
                                      w1sw[:, d0 * F:(d0 + 1) * F])
            else:
                _half = ND * F // 2
                nc.sync.dma_start(w1_all[:, :_half], w1sw[:, :_half])
                nc.sync.dma_start(w1_all[:, _half:], w1sw[:, _half:])
            c1_sb = wts.tile([128, D + NF + NTG], f32, name="c1_sb")
            nc.sync.dma_start(c1_sb[:], cst1[:])
            b2_sb = c1_sb[:, 0:D]
            b1_sb = c1_sb[:, D:D + NF]
            wg_sb = c1_sb[:, D + NF:D + NF + NTG]
            scat_sb = wts.tile([128, NTG], i32, name="scat_sb")
            nc.sync.dma_start(scat_sb[:], scat[:])
            eps_sb = wts.tile([128, 1], f32, name="eps_sb")
            nc.vector.memset(eps_sb[:], LN_EPS)
            zero_sb = wts.tile([128, 2 * D], bf16, name="zero_sb")
            nc.vector.memset(zero_sb[:], 0.0)

            partial = dram.tile([N, D], bf16, name="partial")
            partial_r = partial.rearrange("(t p) d -> t p d", p=128)
            partial_r2 = partial.rearrange("(t u p) d -> t u p d", u=2, p=128)
            if do_memset:
                for t in range(NT):
                    nc.sync.dma_start(partial_r[t], zero_sb[:, :D])

            w2_all = wts.tile([128, NF * D], bf16, name="w2_all")
            if unmerged:
                for f0 in range(NF):
                    nc.sync.dma_start(w2_all[:, f0 * D:(f0 + 1) * D],
                                      w2sw[:, f0 * D:(f0 + 1) * D])
            else:
                _half2 = NF * D // 2
                nc.sync.dma_start(w2_all[:, :_half2], w2sw[:, :_half2])
                nc.sync.dma_start(w2_all[:, _half2:], w2sw[:, _half2:])

            with tc.tile_pool(name="htp", bufs=1) as ht_pool:
                tok0 = 0
                for blk, tb in enumerate(GBLOCKS):
                    if blk == 0:
                        xb = xb0
                    else:
                        xb = xs_pool.tile([128, ND * 512], bf16, name="xb",
                                          tag="xb")
                        if unmerged:
                            for d0 in range(ND):
                                nc.sync.dma_start(
                                    xb[:, d0 * tb:(d0 + 1) * tb],
                                    xgsw[:, blk_off[blk] + d0 * tb:
                                         blk_off[blk] + (d0 + 1) * tb])
                        else:
                            nc.sync.dma_start(
                                xb[:, :ND * tb],
                                xgsw[:, blk_off[blk]:blk_off[blk + 1]])
                    ht = []
                    for f0 in range(NF):
                        hp = psum_h.tile([128, 512], f32, name="hp", tag="hp")
                        for d0 in range(ND):
                            nc.tensor.matmul(
                                hp[:, :tb],
                                lhsT=w1_all[:, d0 * F + f0 * 128:
                                            d0 * F + (f0 + 1) * 128],
                                rhs=xb[:, d0 * tb:(d0 + 1) * tb],
                                start=(d0 == 0),
                                stop=(d0 == ND - 1),
                            )
                        hs = ht_pool.tile([128, 512], bf16, name=f"ht{f0}",
                                          tag=f"ht{f0}")
                        nc.scalar.activation(
                            hs[:, :tb], hp[:, :tb], Act.Relu,
                            bias=b1_sb[:, f0:f0 + 1]
                        )
                        ht.append(hs)
                    for ts in range(tb // 128):
                        j = tok0 // 128 + ts
                        op0 = psum_o.tile([128, 512], f32, name="op0", tag="op")
                        op1 = psum_o.tile([128, 512], f32, name="op1", tag="op")
                        for f0 in range(NF):
                            nc.tensor.matmul(
                                op0[:],
                                lhsT=ht[f0][:, ts * 128:(ts + 1) * 128],
                                rhs=w2_all[:, f0 * D:f0 * D + 512],
                                start=(f0 == 0),
                                stop=(f0 == NF - 1),
                            )
                            nc.tensor.matmul(
                                op1[:],
                                lhsT=ht[f0][:, ts * 128:(ts + 1) * 128],
                                rhs=w2_all[:, f0 * D + 512:f0 * D + 1024],
                                start=(f0 == 0),
                                stop=(f0 == NF - 1),
                            )
                        stb = stb_pool.tile([128, D], bf16, name="stb", tag="stb")
                        for dn, op_ in enumerate((op0, op1)):
                            stf = stg_pool.tile([128, 512], f32, name="stf",
                                                tag="stf")
                            nc.vector.tensor_tensor(
                                stf[:], op_[:], b2_sb[:, dn * 512:(dn + 1) * 512],
                                op=Alu.add,
                            )
                            nc.vector.tensor_scalar_mul(
                                stb[:, dn * 512:(dn + 1) * 512], stf[:],
                                wg_sb[:, j:j + 1]
                            )
                        if do_scatter:
                            nc.gpsimd.indirect_dma_start(
                                out=partial[:],
                                out_offset=IndirectOffsetOnAxis(
                                    ap=scat_sb[:, j:j + 1], axis=0
                                ),
                                in_=stb[:],
                                in_offset=None,
                                bounds_check=N - 1,
                                oob_is_err=False,
                            )
                        else:
                            nc.sync.dma_start(partial_r[j], stb[:])
                    tok0 += tb

                rs_out = dram.tile([TSLICE, D], bf16, name="rs_out")
                if do_collective:
                    nc.gpsimd.collective_compute(
                        "ReduceScatter",
                        Alu.add,
                        replica_groups=[list(range(NC))],
                        ins=[partial.opt()],
                        outs=[rs_out.opt()],
                    )
            rs_r = rs_out.rearrange("(t p) d -> t p d", p=128)

            if debug_partial:
                with tc.tile_pool(name="dbg", bufs=2) as dbg_pool:
                    for i in range(TSLICE // 128):
                        dsb = dbg_pool.tile([128, D], bf16, name="dsb", tag="d")
                        nc.sync.dma_start(dsb[:], partial_r[i])
                        dsf = dbg_pool.tile([128, D], f32, name="dsf", tag="df")
                        nc.vector.tensor_copy(dsf[:], dsb[:])
                        nc.sync.dma_start(out_r[i], dsf[:])
            with tc.tile_pool(name="ln", bufs=1) as ln_pool:
                if debug_partial:
                    ln_pool  # placeholder; LN skipped in debug mode
                if debug_partial:
                    c2_sb = None
                else:
                    c2_sb = ln_pool.tile([128, 2 * D], f32, name="c2_sb", tag="c2")
                    nc.sync.dma_start(c2_sb[:], cst2[:])
                gm_sb = None if debug_partial else c2_sb[:, 0:D]
                bt_sb = None if debug_partial else c2_sb[:, D:2 * D]
                for i in range(0 if debug_partial else TSLICE // 128):
                    rs_sb = ln_pool.tile([128, D], bf16, name="rs_sb", tag="rs")
                    nc.sync.dma_start(rs_sb[:], rs_r[i])
                    xr = ln_pool.tile([128, D], f32, name="xr", tag="xr")
                    nc.sync.dma_start(xr[:], xres_r[i])
                    y = ln_pool.tile([128, D], f32, name="y", tag="y")
                    nc.vector.tensor_tensor(y[:], rs_sb[:], xr[:], op=Alu.add)
                    mu = ln_pool.tile([128, 1], f32, name="mu", tag="mu")
                    nc.vector.reduce_sum(mu[:], y[:], axis=AX)
                    nc.vector.tensor_scalar_mul(mu[:], mu[:], 1.0 / D)
                    xc = ln_pool.tile([128, D], f32, name="xc", tag="xc")
                    nc.vector.tensor_scalar_sub(xc[:], y[:], mu[:])
                    sq = ln_pool.tile([128, D], f32, name="sq", tag="sq")
                    var = ln_pool.tile([128, 1], f32, name="var", tag="var")
                    nc.scalar.activation(sq[:], xc[:], Act.Square,
                                         accum_out=var[:])
                    std = ln_pool.tile([128, 1], f32, name="std", tag="std")
                    nc.scalar.activation(std[:], var[:], Act.Sqrt,
                                         scale=1.0 / D, bias=eps_sb[:])
                    rstd = ln_pool.tile([128, 1], f32, name="rstd", tag="rstd")
                    nc.vector.reciprocal(rstd[:], std[:])
                    o_sb = ln_pool.tile([128, D], f32, name="o_sb", tag="o")
                    nc.vector.scalar_tensor_tensor(
                        o_sb[:], in0=xc[:], scalar=rstd[:], in1=gm_sb,
                        op0=Alu.mult, op1=Alu.mult,
                    )
                    nc.vector.tensor_tensor(o_sb[:], o_sb[:], bt_sb,
                                            op=Alu.add)
                    nc.sync.dma_start(out_r[i], o_sb[:])

    nc.finalize()
    return nc


C2 = 176                # capacity per (expert, owner) segment (mean ~128)
C3 = C2 * NC            # 1408 gathered tokens per expert
NTG3 = C3 // 128        # 11 gathered tiles
C2A, C2B = 96, 80       # A2A split: first/last rows of each segment
SPLA, SPLB = C2A * NC, C2B * NC      # 768 / 640 rows
NTA, NTB = SPLA // 128, SPLB // 128  # 6 / 5 tiles
GBLOCKS3 = [512, 256, 512, 128]      # blocks 0-1 fill A, 2-3 fill B


def _build_nc_sparse3():
    """v3: top-2 sparse with AllToAll combine.

    Gathered tokens for expert e are laid out as 8 owner segments of C2
    rows (tokens owned by core r, padded). Each core computes its expert's
    weighted outputs into a2a_in [C3, D] bf16 with plain DMAs, AllToAll
    swaps owner segments, and each core then sums the 8 received segments
    into its own 512-token slice with a one-hot matmul on PE, fused with
    residual + LayerNorm.
    """
    import concourse.bacc as bacc
    import concourse.mybir as mybir
    import concourse.tile as tile

    dt = mybir.dt
    f32, bf16, i32 = dt.float32, dt.bfloat16, dt.int32
    Alu = mybir.AluOpType
    Act = mybir.ActivationFunctionType
    AX = mybir.AxisListType.X

    nc = bacc.Bacc(num_devices=NC)

    xgsw = nc.dram_tensor("xgsw", [128, ND * C3], bf16, kind="ExternalInput")
    w1sw = nc.dram_tensor("w1sw", [128, ND * F], bf16, kind="ExternalInput")
    w2sw = nc.dram_tensor("w2sw", [128, NF * D], bf16, kind="ExternalInput")
    cst1 = nc.dram_tensor("cst1", [128, D + NF + NTG3], f32, kind="ExternalInput")
    # cst2: gamma_rep | beta_rep | iota row 0..511
    cst2 = nc.dram_tensor("cst2", [128, 2 * D + 512], f32, kind="ExternalInput")
    locs = nc.dram_tensor("locs", [128, NTG3], i32, kind="ExternalInput")
    xres = nc.dram_tensor("xres", [TSLICE, D], f32, kind="ExternalInput")
    out = nc.dram_tensor("out", [TSLICE, D], f32, kind="ExternalOutput")

    xres_r = xres.ap().rearrange("(a p) d -> a p d", p=128)
    out_r = out.ap().rearrange("(a p) d -> a p d", p=128)

    blk_off = [0]
    for tb in GBLOCKS3:
        blk_off.append(blk_off[-1] + ND * tb)

    with tile.TileContext(nc) as tc:
        with (
            tc.tile_pool(name="wts", bufs=1) as wts,
            tc.tile_pool(name="xs", bufs=1) as xs_pool,
            tc.tile_pool(name="stg", bufs=2) as stg_pool,
            tc.tile_pool(name="stb", bufs=3) as stb_pool,
            tc.tile_pool(name="psh", bufs=2, space="PSUM") as psum_h,
            tc.tile_pool(name="pso", bufs=6, space="PSUM") as psum_o,
            tc.tile_pool(name="dram", bufs=1, space="DRAM") as dram,
        ):
            xb0 = xs_pool.tile([128, ND * 512], bf16, name="xb0", tag="xb")
            nc.sync.dma_start(xb0[:, :ND * GBLOCKS3[0]],
                              xgsw[:, blk_off[0]:blk_off[1]])
            w1_all = wts.tile([128, ND * F], bf16, name="w1_all")
            _half = ND * F // 2
            nc.sync.dma_start(w1_all[:, :_half], w1sw[:, :_half])
            nc.sync.dma_start(w1_all[:, _half:], w1sw[:, _half:])
            c1_sb = wts.tile([128, D + NF + NTG3], f32, name="c1_sb")
            nc.sync.dma_start(c1_sb[:], cst1[:])
            b2_sb = c1_sb[:, 0:D]
            b1_sb = c1_sb[:, D:D + NF]
            wg_sb = c1_sb[:, D + NF:D + NF + NTG3]
            locs_sb = wts.tile([128, NTG3], i32, name="locs_sb")
            nc.sync.dma_start(locs_sb[:], locs[:])
            eps_sb = wts.tile([128, 1], f32, name="eps_sb")
            nc.vector.memset(eps_sb[:], LN_EPS)

            a2a_inA = dram.tile([SPLA, D], bf16, name="a2a_inA")
            a2a_inA_r = a2a_inA.rearrange("(t p) d -> t p d", p=128)
            a2a_inB = dram.tile([SPLB, D], bf16, name="a2a_inB")
            a2a_inB_r = a2a_inB.rearrange("(t p) d -> t p d", p=128)
            a2a_outA = dram.tile([SPLA, D], bf16, name="a2a_outA")
            a2a_outB = dram.tile([SPLB, D], bf16, name="a2a_outB")

            w2_all = wts.tile([128, NF * D], bf16, name="w2_all")
            _half2 = NF * D // 2
            nc.sync.dma_start(w2_all[:, :_half2], w2sw[:, :_half2])
            nc.sync.dma_start(w2_all[:, _half2:], w2sw[:, _half2:])

            with tc.tile_pool(name="htp", bufs=1) as ht_pool:
                tok0 = 0
                for blk, tb in enumerate(GBLOCKS3):
                    if blk == 0:
                        xb = xb0
                    else:
                        xb = xs_pool.tile([128, ND * 512], bf16, name="xb",
                                          tag="xb")
                        nc.sync.dma_start(
                            xb[:, :ND * tb],
                            xgsw[:, blk_off[blk]:blk_off[blk + 1]])
                    ht = []
                    for f0 in range(NF):
                        hp = psum_h.tile([128, 512], f32, name="hp", tag="hp")
                        for d0 in range(ND):
                            nc.tensor.matmul(
                                hp[:, :tb],
                                lhsT=w1_all[:, d0 * F + f0 * 128:
                                            d0 * F + (f0 + 1) * 128],
                                rhs=xb[:, d0 * tb:(d0 + 1) * tb],
                                start=(d0 == 0),
                                stop=(d0 == ND - 1),
                            )
                        hs = ht_pool.tile([128, 512], bf16, name=f"ht{f0}",
                                          tag=f"ht{f0}")
                        nc.scalar.activation(
                            hs[:, :tb], hp[:, :tb], Act.Relu,
                            bias=b1_sb[:, f0:f0 + 1]
                        )
                        ht.append(hs)
                    for ts in range(tb // 128):
                        j = tok0 // 128 + ts
                        op0 = psum_o.tile([128, 512], f32, name="op0", tag="op")
                        op1 = psum_o.tile([128, 512], f32, name="op1", tag="op")
                        for f0 in range(NF):
                            nc.tensor.matmul(
                                op0[:],
                                lhsT=ht[f0][:, ts * 128:(ts + 1) * 128],
                                rhs=w2_all[:, f0 * D:f0 * D + 512],
                                start=(f0 == 0),
                                stop=(f0 == NF - 1),
                            )
                            nc.tensor.matmul(
                                op1[:],
                                lhsT=ht[f0][:, ts * 128:(ts + 1) * 128],
                                rhs=w2_all[:, f0 * D + 512:f0 * D + 1024],
                                start=(f0 == 0),
                                stop=(f0 == NF - 1),
                            )
                        stb = stb_pool.tile([128, D], bf16, name="stb", tag="stb")
                        for dn, op_ in enumerate((op0, op1)):
                            stf = stg_pool.tile([128, 512], f32, name="stf",
                                                tag="stf")
                            nc.vector.tensor_tensor(
                                stf[:], op_[:], b2_sb[:, dn * 512:(dn + 1) * 512],
                                op=Alu.add,
                            )
                            nc.vector.tensor_scalar_mul(
                                stb[:, dn * 512:(dn + 1) * 512], stf[:],
                                wg_sb[:, j:j + 1]
                            )
                        if j < NTA:
                            nc.sync.dma_start(a2a_inA_r[j], stb[:])
                        else:
                            nc.sync.dma_start(a2a_inB_r[j - NTA], stb[:])
                    tok0 += tb
                    if blk == 1:
                        nc.gpsimd.collective_compute(
                            "AllToAll",
                            Alu.bypass,
                            replica_groups=[list(range(NC))],
                            ins=[a2a_inA.opt()],
                            outs=[a2a_outA.opt()],
                        )
                nc.gpsimd.collective_compute(
                    "AllToAll",
                    Alu.bypass,
                    replica_groups=[list(range(NC))],
                    ins=[a2a_inB.opt()],
                    outs=[a2a_outB.opt()],
                )
            a2a_outA_r = a2a_outA.rearrange("(t p) d -> t p d", p=128)
            a2a_outB_r = a2a_outB.rearrange("(t p) d -> t p d", p=128)

            # --- combine received segments into own slice + residual + LN ---
            with tc.tile_pool(name="cmb", bufs=1) as cmb_pool:
                c2_sb = cmb_pool.tile([128, 2 * D + 512], f32, name="c2_sb",
                                      tag="c2")
                nc.sync.dma_start(c2_sb[:], cst2[:])
                gm_sb = c2_sb[:, 0:D]
                bt_sb = c2_sb[:, D:2 * D]
                iota_sb = c2_sb[:, 2 * D:2 * D + 512]
                liv = cmb_pool.tile([128, NTG3], f32, name="liv", tag="liv")
                nc.vector.tensor_copy(liv[:], locs_sb[:])
                # 8 PSUM banks accumulate all (mt, dn) outputs across kt
                pst = []
                for mt in range(TSLICE // 128):
                    pa = psum_o.tile([128, 512], f32, name=f"cA{mt}", tag="op")
                    pb = (psum_h if mt >= 2 else psum_o).tile(
                        [128, 512], f32, name=f"cB{mt}",
                        tag="hp" if mt >= 2 else "op")
                    pst.append((pa, pb))
                for kt in range(NTG3):
                    s = cmb_pool.tile([128, 512], bf16, name="sel", tag="sel",
                                      bufs=2)
                    nc.vector.tensor_scalar(
                        s[:], iota_sb, liv[:, kt:kt + 1], None,
                        op0=Alu.is_equal,
                    )
                    rt = cmb_pool.tile([128, D], bf16, name="arow", tag="arow",
                                       bufs=2)
                    if kt < NTA:
                        nc.sync.dma_start(rt[:], a2a_outA_r[kt])
                    else:
                        nc.sync.dma_start(rt[:], a2a_outB_r[kt - NTA])
                    for mt in range(TSLICE // 128):
                        nc.tensor.matmul(
                            pst[mt][0][:],
                            lhsT=s[:, mt * 128:(mt + 1) * 128],
                            rhs=rt[:, 0:512],
                            start=(kt == 0),
                            stop=(kt == NTG3 - 1),
                        )
                        nc.tensor.matmul(
                            pst[mt][1][:],
                            lhsT=s[:, mt * 128:(mt + 1) * 128],
                            rhs=rt[:, 512:1024],
                            start=(kt == 0),
                            stop=(kt == NTG3 - 1),
                        )
                for mt in range(TSLICE // 128):
                    psA, psB = pst[mt]
                    xr = cmb_pool.tile([128, D], f32, name="xr", tag="xr")
                    nc.sync.dma_start(xr[:], xres_r[mt])
                    y = cmb_pool.tile([128, D], f32, name="y", tag="y")
                    nc.vector.tensor_tensor(y[:, 0:512], psA[:], xr[:, 0:512],
                                            op=Alu.add)
                    nc.vector.tensor_tensor(y[:, 512:1024], psB[:],
                                            xr[:, 512:1024], op=Alu.add)
                    mu = cmb_pool.tile([128, 1], f32, name="mu", tag="mu")
                    nc.vector.reduce_sum(mu[:], y[:], axis=AX)
                    nc.vector.tensor_scalar_mul(mu[:], mu[:], 1.0 / D)
                    xc = cmb_pool.tile([128, D], f32, name="xc", tag="xc")
                    nc.vector.tensor_scalar_sub(xc[:], y[:], mu[:])
                    sq = cmb_pool.tile([128, D], f32, name="sq", tag="sq")
                    var = cmb_pool.tile([128, 1], f32, name="var", tag="var")
                    nc.scalar.activation(sq[:], xc[:], Act.Square,
                                         accum_out=var[:])
                    std = cmb_pool.tile([128, 1], f32, name="std", tag="std")
                    nc.scalar.activation(std[:], var[:], Act.Sqrt,
                                         scale=1.0 / D, bias=eps_sb[:])
                    rstd = cmb_pool.tile([128, 1], f32, name="rstd", tag="rstd")
                    nc.vector.reciprocal(rstd[:], std[:])
                    o_sb = cmb_pool.tile([128, D], f32, name="o_sb", tag="o")
                    nc.vector.scalar_tensor_tensor(
                        o_sb[:], in0=xc[:], scalar=rstd[:], in1=gm_sb,
                        op0=Alu.mult, op1=Alu.mult,
                    )
                    nc.vector.tensor_tensor(o_sb[:], o_sb[:], bt_sb,
                                            op=Alu.add)
                    nc.sync.dma_start(out_r[mt], o_sb[:])

    nc.finalize()
    return nc


def _build_in_maps_sparse3(tgt, Wr, br, W1, b1, W2, b2, gamma, beta):
    """Segment-padded gather for the AllToAll combine. Returns None when any
    (expert, owner) segment exceeds C2 (caller falls back)."""
    f32 = np.float32
    x = np.ascontiguousarray(np.asarray(tgt, f32).reshape(N, D))
    Wr = np.asarray(Wr, f32)
    br = np.asarray(br, f32)
    W1 = np.asarray(W1, f32)
    b1 = np.asarray(b1, f32)
    W2 = np.asarray(W2, f32)
    b2 = np.asarray(b2, f32)
    gamma = np.asarray(gamma, f32)
    beta = np.asarray(beta, f32)

    i1, i2, w1, w2 = _route_host(x, Wr, br)
    iota = np.broadcast_to(np.arange(512, dtype=f32), (128, 512))
    cst2 = np.ascontiguousarray(np.concatenate([
        np.broadcast_to(gamma, (128, D)),
        np.broadcast_to(beta, (128, D)),
        iota,
    ], axis=1))

    xt = x.T  # [D, N] view

    # per (expert, owner) token lists, split into A (first C2A) / B (rest)
    all_idx = []
    all_w = []
    all_loc = []
    for e in range(NC):
        sel = (i1 == e) | (i2 == e)
        idx_e = np.nonzero(sel)[0]
        w_e = np.where(i1[idx_e] == e, w1[idx_e], w2[idx_e]).astype(f32)
        seg_idx = np.zeros(C3, np.int64)
        seg_w = np.zeros(C3, f32)
        seg_loc = np.full(C3, PAD_IDX, np.int64)
        for r in range(NC):
            m = (idx_e >= r * TSLICE) & (idx_e < (r + 1) * TSLICE)
            li = idx_e[m]
            wi = w_e[m]
            if li.size > C2:
                return None
            na = min(li.size, C2A)
            a0 = r * C2A
            seg_idx[a0:a0 + na] = li[:na]
            seg_w[a0:a0 + na] = wi[:na]
            seg_loc[a0:a0 + na] = li[:na] - r * TSLICE
            nb = li.size - na
            if nb > 0:
                b0 = SPLA + r * C2B
                seg_idx[b0:b0 + nb] = li[na:]
                seg_w[b0:b0 + nb] = wi[na:]
                seg_loc[b0:b0 + nb] = li[na:] - r * TSLICE
        all_idx.append(seg_idx)
        all_w.append(seg_w)
        all_loc.append(seg_loc)

    in_maps = []
    for e in range(NC):
        seg_idx, seg_w = all_idx[e], all_w[e]
        xg = xt[:, seg_idx].reshape(ND, 128, C3)
        parts = []
        c0 = 0
        for tb in GBLOCKS3:
            parts.append(xg[:, :, c0:c0 + tb].transpose(1, 0, 2)
                         .reshape(128, ND * tb))
            c0 += tb
        xgsw = np.ascontiguousarray(np.concatenate(parts, axis=1)).astype(BF16)
        w1sw = np.ascontiguousarray(
            W1[e].T.reshape(ND, 128, F).transpose(1, 0, 2).reshape(128, ND * F)
        ).astype(BF16)
        w2sw = np.ascontiguousarray(
            W2[e].T.reshape(NF, 128, D).transpose(1, 0, 2).reshape(128, NF * D)
        ).astype(BF16)
        cst1 = np.ascontiguousarray(np.concatenate([
            np.broadcast_to(b2[e], (128, D)),
            b1[e].reshape(NF, 128).T,
            seg_w.reshape(NTG3, 128).T,
        ], axis=1))
        # locs for the RECEIVER side: core e receives segment e from every
        # expert core s: rows [s*C2, (s+1)*C2) of its a2a_out = expert s's
        # tokens owned by core e -> local idx = all_loc[s][e*C2 + k]
        loc_rows = np.full(C3, PAD_IDX, np.int64)
        for s in range(NC):
            loc_rows[s * C2A:(s + 1) * C2A] = \
                all_loc[s][e * C2A:(e + 1) * C2A]
            loc_rows[SPLA + s * C2B:SPLA + (s + 1) * C2B] = \
                all_loc[s][SPLA + e * C2B:SPLA + (e + 1) * C2B]
        locs = np.ascontiguousarray(
            loc_rows.astype(np.int32).reshape(NTG3, 128).T)
        in_maps.append({
            "xgsw": xgsw,
            "w1sw": w1sw,
            "w2sw": w2sw,
            "cst1": cst1,
            "cst2": cst2,
            "locs": locs,
            "xres": np.ascontiguousarray(x[e * TSLICE:(e + 1) * TSLICE]),
        })
    return in_maps


def _get_nc(kind="dense"):
    key = f"nc_{kind}"
    if key not in _CACHE:
        builders = {"dense": _build_nc, "sparse": _build_nc_sparse,
                    "sparse3": _build_nc_sparse3}
        _CACHE[key] = builders[kind]()
    return _CACHE[key]


def _route_host(x, Wr, br):
    """fp32 router: returns (i1, i2, w1, w2) top-2 indices and renormalized
    softmax weights per token."""
    logits = x @ Wr.T.astype(np.float32) + br.astype(np.float32)  # [N, E]
    i1 = np.argmax(logits, axis=1)
    v1 = logits[np.arange(N), i1]
    masked = logits.copy()
    masked[np.arange(N), i1] = -np.inf
    i2 = np.argmax(masked, axis=1)
    v2 = masked[np.arange(N), i2]
    ew = np.exp(v2 - v1)            # <= 1
    w1 = 1.0 / (1.0 + ew)
    w2 = ew / (1.0 + ew)
    return i1, i2, w1.astype(np.float32), w2.astype(np.float32)


def _build_in_maps_sparse(tgt, Wr, br, W1, b1, W2, b2, gamma, beta):
    """Returns None if any expert's token count exceeds CCAP (caller falls
    back to the dense kernel)."""
    f32 = np.float32
    x = np.ascontiguousarray(np.asarray(tgt, f32).reshape(N, D))
    Wr = np.asarray(Wr, f32)
    br = np.asarray(br, f32)
    W1 = np.asarray(W1, f32)
    b1 = np.asarray(b1, f32)
    W2 = np.asarray(W2, f32)
    b2 = np.asarray(b2, f32)
    gamma = np.asarray(gamma, f32)
    beta = np.asarray(beta, f32)

    i1, i2, w1, w2 = _route_host(x, Wr, br)
    cst2 = np.ascontiguousarray(np.concatenate([
        np.broadcast_to(gamma, (128, D)),
        np.broadcast_to(beta, (128, D)),
    ], axis=1))

    xt = x.T  # [D, N] view

    in_maps = []
    for e in range(NC):
        sel1 = i1 == e
        sel2 = i2 == e
        idx = np.nonzero(sel1 | sel2)[0]
        if idx.size > CCAP:
            return None
        w = np.where(sel1[idx], w1[idx], w2[idx]).astype(f32)
        idx_pad = np.zeros(CCAP, np.int64)
        idx_pad[:idx.size] = idx
        scat_ids = np.full(CCAP, PAD_IDX, np.int32)
        scat_ids[:idx.size] = idx.astype(np.int32)
        wg = np.zeros(CCAP, f32)
        wg[:idx.size] = w

        # xgsw [128, ND*CCAP]: block-major columns, then d-tile, then token
        xg = xt[:, idx_pad].reshape(ND, 128, CCAP)   # [d0, p, c]
        parts = []
        c0 = 0
        for tb in GBLOCKS:
            parts.append(xg[:, :, c0:c0 + tb].transpose(1, 0, 2).reshape(128, ND * tb))
            c0 += tb
        xgsw = np.ascontiguousarray(np.concatenate(parts, axis=1)).astype(BF16)

        w1sw = np.ascontiguousarray(
            W1[e].T.reshape(ND, 128, F).transpose(1, 0, 2).reshape(128, ND * F)
        ).astype(BF16)
        w2sw = np.ascontiguousarray(
            W2[e].T.reshape(NF, 128, D).transpose(1, 0, 2).reshape(128, NF * D)
        ).astype(BF16)
        cst1 = np.ascontiguousarray(np.concatenate([
            np.broadcast_to(b2[e], (128, D)),
            b1[e].reshape(NF, 128).T,
            wg.reshape(NTG, 128).T,
        ], axis=1))
        in_maps.append({
            "xgsw": xgsw,
            "w1sw": w1sw,
            "w2sw": w2sw,
            "cst1": cst1,
            "cst2": cst2,
            "scat": np.ascontiguousarray(scat_ids.reshape(NTG, 128).T),
            "xres": np.ascontiguousarray(x[e * TSLICE:(e + 1) * TSLICE]),
        })
    return in_maps


def _build_in_maps(tgt, Wr, br, W1, b1, W2, b2, gamma, beta):
    f32 = np.float32
    tgt = np.asarray(tgt, dtype=f32)
    Wr = np.asarray(Wr, dtype=f32)
    br = np.asarray(br, dtype=f32)
    W1 = np.asarray(W1, dtype=f32)
    b1 = np.asarray(b1, dtype=f32)
    W2 = np.asarray(W2, dtype=f32)
    b2 = np.asarray(b2, dtype=f32)
    gamma = np.asarray(gamma, dtype=f32)
    beta = np.asarray(beta, dtype=f32)

    x = np.ascontiguousarray(tgt.reshape(N, D))
    xt = np.ascontiguousarray(x.T)                      # [D, N] fp32
    xtb = xt.astype(BF16)
    gmr = np.ascontiguousarray(np.broadcast_to(gamma, (128, D)))
    btr = np.ascontiguousarray(np.broadcast_to(beta, (128, D)))

    in_maps = []
    for e in range(NC):
        perm = [e] + [i for i in range(E) if i != e]
        in_maps.append({
            "xtf": xt,
            "xtb": xtb,
            "w1t": np.ascontiguousarray(W1[e].T).astype(BF16),   # [D, F]
            "w2t": np.ascontiguousarray(W2[e].T).astype(BF16),   # [F, D]
            "b1c": np.ascontiguousarray(b1[e].reshape(NF, 128).T),  # [128, NF]
            "b2r": np.ascontiguousarray(np.broadcast_to(b2[e], (128, D))),
            "wrt": np.ascontiguousarray(Wr[perm].T),             # [D, E]
            "brr": np.ascontiguousarray(np.broadcast_to(br[perm], (128, E))),
            "xres": np.ascontiguousarray(x[e * TSLICE:(e + 1) * TSLICE]),
            "gmr": gmr,
            "btr": btr,
        })
    return in_maps


def _assemble(results):
    out = np.concatenate([results[r]["out"] for r in range(NC)], axis=0)
    return np.ascontiguousarray(out.reshape(B, S, D).astype(np.float32))


def _get_runner(kind="dense"):
    """Build (once per kind) a cached jitted SPMD executor mirroring
    concourse.bass2jax.run_bass_via_pjrt, so repeat kernel() calls skip
    recompilation."""
    rkey = f"runner_{kind}"
    if rkey in _CACHE:
        return _CACHE[rkey]

    import jax
    from jax.sharding import Mesh, PartitionSpec
    from jax.experimental.shard_map import shard_map
    import concourse.mybir as mybir
    from concourse import bass2jax

    nc = _get_nc(kind)
    bass2jax.install_neuronx_cc_hook()

    partition_name = nc.partition_id_tensor.name if nc.partition_id_tensor else None
    in_names, out_names, out_avals, zero_outs = [], [], [], []
    for alloc in nc.m.functions[0].allocations:
        if not isinstance(alloc, mybir.MemoryLocationSet):
            continue
        name = alloc.memorylocations[0].name
        if alloc.kind == "ExternalInput":
            if name != partition_name:
                in_names.append(name)
        elif alloc.kind == "ExternalOutput":
            shape = tuple(alloc.tensor_shape)
            dtype = mybir.dt.np(alloc.dtype)
            out_names.append(name)
            out_avals.append(jax.core.ShapedArray(shape, dtype))
            zero_outs.append(np.zeros(shape, dtype))
    n_params = len(in_names)
    n_outs = len(out_avals)
    all_in_names = list(in_names) + list(out_names)
    if partition_name is not None:
        all_in_names.append(partition_name)

    donate = tuple(range(n_params, n_params + n_outs))

    def _body(*args):
        operands = list(args)
        if partition_name is not None:
            operands.append(bass2jax.partition_id_tensor())
        outs = bass2jax._bass_exec_p.bind(
            *operands,
            out_avals=tuple(out_avals),
            in_names=tuple(all_in_names),
            out_names=tuple(out_names),
            lowering_input_output_aliases=(),
            sim_require_finite=True,
            sim_require_nnan=True,
            nc=nc,
        )
        return tuple(outs)

    devices = jax.devices()[:NC]
    mesh = Mesh(np.asarray(devices), ("core",))
    in_specs = (PartitionSpec("core"),) * (n_params + n_outs)
    out_specs = (PartitionSpec("core"),) * n_outs
    sharded = jax.jit(
        shard_map(
            _body, mesh=mesh, in_specs=in_specs, out_specs=out_specs,
            check_rep=False,
        ),
        donate_argnums=donate,
        keep_unused=True,
    )

    def run(in_maps):
        per_core = [[np.asarray(m[name]) for name in in_names] for m in in_maps]
        concat_in = [
            np.concatenate([per_core[c][i] for c in range(NC)], axis=0)
            for i in range(n_params)
        ]
        concat_zeros = [
            np.zeros((NC * z.shape[0], *z.shape[1:]), z.dtype) for z in zero_outs
        ]
        out_arrs = sharded(*concat_in, *concat_zeros)
        return [
            {
                name: np.asarray(out_arrs[i]).reshape(NC, *out_avals[i].shape)[c]
                for i, name in enumerate(out_names)
            }
            for c in range(NC)
        ]

    _CACHE[rkey] = run
    return run


def kernel(tgt, Wr, br, W1, b1, W2, b2, gamma, beta):
    args = (tgt, Wr, br, W1, b1, W2, b2, gamma, beta)
    # fastest path: per-(expert, owner)-segment capacity C2
    try:
        in_maps = _build_in_maps_sparse3(*args)
    except Exception:
        in_maps = None
    if in_maps is not None:
        try:
            results = _get_runner("sparse3")(in_maps)
            return _assemble(results)
        except Exception:
            pass
    # second chance: per-expert capacity CCAP (tolerates skewed owners)
    try:
        in_maps = _build_in_maps_sparse(*args)
    except Exception:
        in_maps = None
    if in_maps is not None:
        try:
            results = _get_runner("sparse")(in_maps)
            return _assemble(results)
        except Exception:
            pass
    # Dense fallback: capacity overflow or any sparse-path failure.
    in_maps = _build_in_maps(*args)
    try:
        results = _get_runner("dense")(in_maps)
    except Exception:
        # Last resort: the well-tested (but recompiling) library path.
        from concourse.bass_utils import run_bass_kernel_spmd
        res = run_bass_kernel_spmd(
            _get_nc("dense"), in_maps, core_ids=list(range(NC))
        )
        results = res.results
    return _assemble(results)



# revision 5
# speedup vs baseline: 1.3826x; 1.3826x over previous
"""Trainium2 Bass kernel for a dense-MoE FFN layer (top-2 routing).

Expert-parallel over 8 NeuronCores: core e owns expert e (W1[e], W2[e]).
Every core:
  - computes fp32 router logits for all tokens (replicated router),
    derives its own expert's per-token top-2 softmax weight on device,
  - runs the dense expert FFN in bf16 (fp32 accumulate in PSUM),
  - scales by the router weight, writes a partial sum [N, D],
  - ReduceScatter(+) over the 8 cores -> each core holds the summed
    MoE output for a distinct 512-token slice,
  - adds the residual and applies LayerNorm on that slice.
The host concatenates the 8 slices into the full [B, S, D] output.
"""

import numpy as np
import ml_dtypes

B, S, D, F, E = 2, 2048, 1024, 4096, 8
N = B * S              # 4096 tokens
NC = 8                 # cores
TSLICE = N // NC       # 512 tokens output slice per core
TB = 512               # token block for the matmul pipeline
NB = N // TB           # 8 blocks
ND = D // 128          # 8 d-tiles
NF = F // 128          # 32 f-tiles
NT = N // 128          # 32 token tiles
LN_EPS = 1e-5

BF16 = ml_dtypes.bfloat16

_CACHE = {}


def _build_nc(do_collective=True, n_blocks=NB, do_router=True):
    import concourse.bacc as bacc
    import concourse.mybir as mybir
    import concourse.tile as tile

    dt = mybir.dt
    f32, bf16 = dt.float32, dt.bfloat16
    Alu = mybir.AluOpType
    Act = mybir.ActivationFunctionType
    AX = mybir.AxisListType.X

    nc = bacc.Bacc(num_devices=NC)

    xtf = nc.dram_tensor("xtf", [D, N], f32, kind="ExternalInput")
    xtb = nc.dram_tensor("xtb", [D, N], bf16, kind="ExternalInput")
    w1t = nc.dram_tensor("w1t", [D, F], bf16, kind="ExternalInput")
    w2t = nc.dram_tensor("w2t", [F, D], bf16, kind="ExternalInput")
    b1c = nc.dram_tensor("b1c", [128, NF], f32, kind="ExternalInput")
    b2r = nc.dram_tensor("b2r", [128, D], f32, kind="ExternalInput")
    wrt = nc.dram_tensor("wrt", [D, E], f32, kind="ExternalInput")
    brr = nc.dram_tensor("brr", [128, E], f32, kind="ExternalInput")
    xres = nc.dram_tensor("xres", [TSLICE, D], f32, kind="ExternalInput")
    gmr = nc.dram_tensor("gmr", [128, D], f32, kind="ExternalInput")
    btr = nc.dram_tensor("btr", [128, D], f32, kind="ExternalInput")
    out = nc.dram_tensor("out", [TSLICE, D], f32, kind="ExternalOutput")

    xtf_r = xtf.ap().rearrange("(a p) n -> a p n", p=128)
    xtb_r = xtb.ap().rearrange("(a p) n -> a p n", p=128)
    w1t_r = w1t.ap().rearrange("(a p) f -> a p f", p=128)
    w2t_r = w2t.ap().rearrange("(a p) d -> a p d", p=128)
    wrt_r = wrt.ap().rearrange("(a p) e -> a p e", p=128)
    xres_r = xres.ap().rearrange("(a p) d -> a p d", p=128)
    out_r = out.ap().rearrange("(a p) d -> a p d", p=128)

    with tile.TileContext(nc) as tc:
        with (
            tc.tile_pool(name="wts", bufs=1) as wts,
            tc.tile_pool(name="xs", bufs=1) as xs_pool,
            tc.tile_pool(name="stage", bufs=4) as stage_pool,
            tc.tile_pool(name="psr", bufs=2, space="PSUM") as psum_r,
            tc.tile_pool(name="psh", bufs=2, space="PSUM") as psum_h,
            tc.tile_pool(name="pso", bufs=4, space="PSUM") as psum_o,
            tc.tile_pool(name="dram", bufs=1, space="DRAM") as dram,
        ):
            # --- persistent small tensors ---
            wrt_sb = []
            for d0 in range(ND):
                t = wts.tile([128, E], f32, name=f"wrt{d0}", tag=f"wrt{d0}")
                nc.sync.dma_start(t[:], wrt_r[d0])
                wrt_sb.append(t)
            brr_sb = wts.tile([128, E], f32, name="brr_sb")
            nc.sync.dma_start(brr_sb[:], brr[:])
            b1_sb = wts.tile([128, NF], f32, name="b1_sb")
            nc.sync.dma_start(b1_sb[:], b1c[:])
            b2_sb = wts.tile([128, D], f32, name="b2_sb")
            nc.sync.dma_start(b2_sb[:], b2r[:])
            gm_sb = wts.tile([128, D], f32, name="gm_sb")
            nc.sync.dma_start(gm_sb[:], gmr[:])
            bt_sb = wts.tile([128, D], f32, name="bt_sb")
            nc.sync.dma_start(bt_sb[:], btr[:])
            # per-token router weight for this core's expert, [128, NT]
            w_all = wts.tile([128, NT], f32, name="w_all")
            eps_sb = wts.tile([128, 1], f32, name="eps_sb")
            nc.vector.memset(eps_sb[:], LN_EPS)
            if not do_router:
                nc.vector.memset(w_all[:], 0.5)

            # --- expert weights (persistent, stream in behind the router) ---
            w1_sb = []
            for d0 in range(ND):
                t = wts.tile([128, F], bf16, name=f"w1_{d0}", tag=f"w1_{d0}")
                nc.sync.dma_start(t[:], w1t_r[d0])
                w1_sb.append(t)
            w2_sb = []
            for f0 in range(NF):
                t = wts.tile([128, D], bf16, name=f"w2_{f0}", tag=f"w2_{f0}")
                nc.sync.dma_start(t[:], w2t_r[f0])
                w2_sb.append(t)

            # --- router phase: fp32 logits -> top-2 weight for own expert ---
            with (
                tc.tile_pool(name="xtfp", bufs=2) as xtf_pool,
                tc.tile_pool(name="rtmp", bufs=4) as rtmp,
            ):
                for blk in range(NB if do_router else 0):
                    xf = []
                    for d0 in range(ND):
                        t = xtf_pool.tile([128, TB], f32, name=f"xf{d0}", tag=f"xf{d0}")
                        nc.sync.dma_start(t[:], xtf_r[d0][:, blk * TB:(blk + 1) * TB])
                        xf.append(t)
                    for tt in range(TB // 128):
                        tok = blk * (TB // 128) + tt
                        ps = psum_r.tile([128, E], f32, name="ps_r", tag="ps_r")
                        for d0 in range(ND):
                            nc.tensor.matmul(
                                ps[:],
                                lhsT=xf[d0][:, tt * 128:(tt + 1) * 128],
                                rhs=wrt_sb[d0][:],
                                start=(d0 == 0),
                                stop=(d0 == ND - 1),
                            )
                        lg = rtmp.tile([128, E], f32, name="lg", tag="lg")
                        nc.vector.tensor_tensor(lg[:], ps[:], brr_sb[:], op=Alu.add)
                        m1 = rtmp.tile([128, 1], f32, name="m1", tag="m1")
                        nc.vector.reduce_max(m1[:], lg[:], axis=AX)
                        eq = rtmp.tile([128, E], f32, name="eq", tag="eq")
                        nc.vector.tensor_scalar(
                            eq[:], lg[:], m1[:], None, op0=Alu.is_equal
                        )
                        msk = rtmp.tile([128, E], f32, name="msk", tag="msk")
                        nc.vector.scalar_tensor_tensor(
                            msk[:], in0=eq[:], scalar=-1e30, in1=lg[:],
                            op0=Alu.mult, op1=Alu.add,
                        )
                        m2 = rtmp.tile([128, 1], f32, name="m2", tag="m2")
                        nc.vector.reduce_max(m2[:], msk[:], axis=AX)
                        my = lg[:, 0:1]
                        d1 = rtmp.tile([128, 1], f32, name="d1", tag="d1")
                        nc.vector.tensor_tensor(d1[:], my, m2[:], op=Alu.subtract)
                        d2 = rtmp.tile([128, 1], f32, name="d2", tag="d2")
                        nc.vector.tensor_tensor(d2[:], my, m1[:], op=Alu.subtract)
                        s1 = rtmp.tile([128, 1], f32, name="s1", tag="s1")
                        nc.scalar.activation(s1[:], d1[:], Act.Sigmoid)
                        s2 = rtmp.tile([128, 1], f32, name="s2", tag="s2")
                        nc.scalar.activation(s2[:], d2[:], Act.Sigmoid)
                        e1 = rtmp.tile([128, 1], f32, name="e1", tag="e1")
                        nc.vector.tensor_tensor(e1[:], my, m1[:], op=Alu.is_equal)
                        e2 = rtmp.tile([128, 1], f32, name="e2", tag="e2")
                        nc.vector.tensor_tensor(e2[:], my, m2[:], op=Alu.is_equal)
                        t1 = rtmp.tile([128, 1], f32, name="t1", tag="t1")
                        nc.vector.tensor_tensor(t1[:], e1[:], s1[:], op=Alu.mult)
                        t2 = rtmp.tile([128, 1], f32, name="t2", tag="t2")
                        nc.vector.tensor_tensor(t2[:], e2[:], s2[:], op=Alu.mult)
                        nc.vector.tensor_tensor(
                            w_all[:, tok:tok + 1], t1[:], t2[:], op=Alu.add
                        )

            partial = dram.tile([N, D], f32, name="partial")
            partial_r = partial.rearrange("(t p) d -> t p d", p=128)

            # --- expert FFN blocks ---
            with tc.tile_pool(name="htp", bufs=1) as ht_pool:
                for blk in range(n_blocks):
                    xb = []
                    for d0 in range(ND):
                        t = xs_pool.tile([128, TB], bf16, name=f"xb{d0}", tag=f"xb{d0}")
                        nc.sync.dma_start(t[:], xtb_r[d0][:, blk * TB:(blk + 1) * TB])
                        xb.append(t)
                    # h^T = relu(W1 x^T + b1), produced as 32 [128f, TB] bf16 tiles
                    ht = []
                    for f0 in range(NF):
                        hp = psum_h.tile([128, TB], f32, name="hp", tag="hp")
                        for d0 in range(ND):
                            nc.tensor.matmul(
                                hp[:],
                                lhsT=w1_sb[d0][:, f0 * 128:(f0 + 1) * 128],
                                rhs=xb[d0][:],
                                start=(d0 == 0),
                                stop=(d0 == ND - 1),
                            )
                        hs = ht_pool.tile([128, TB], bf16, name=f"ht{f0}", tag=f"ht{f0}")
                        nc.scalar.activation(
                            hs[:], hp[:], Act.Relu, bias=b1_sb[:, f0:f0 + 1]
                        )
                        ht.append(hs)
                    # o = h W2^T + b2, weighted by router w, to partial DRAM
                    for ts in range(TB // 128):
                        tok = blk * (TB // 128) + ts
                        op0 = psum_o.tile([128, 512], f32, name="op0", tag="op")
                        op1 = psum_o.tile([128, 512], f32, name="op1", tag="op")
                        for f0 in range(NF):
                            nc.tensor.matmul(
                                op0[:],
                                lhsT=ht[f0][:, ts * 128:(ts + 1) * 128],
                                rhs=w2_sb[f0][:, 0:512],
                                start=(f0 == 0),
                                stop=(f0 == NF - 1),
                            )
                            nc.tensor.matmul(
                                op1[:],
                                lhsT=ht[f0][:, ts * 128:(ts + 1) * 128],
                                rhs=w2_sb[f0][:, 512:1024],
                                start=(f0 == 0),
                                stop=(f0 == NF - 1),
                            )
                        for dn, op_ in enumerate((op0, op1)):
                            st = stage_pool.tile([128, 512], f32, name="st", tag="st")
                            nc.vector.tensor_tensor(
                                st[:], op_[:], b2_sb[:, dn * 512:(dn + 1) * 512],
                                op=Alu.add,
                            )
                            nc.vector.tensor_scalar_mul(
                                st[:], st[:], w_all[:, tok:tok + 1]
                            )
                            nc.sync.dma_start(
                                partial_r[tok][:, dn * 512:(dn + 1) * 512], st[:]
                            )

                rs_out = dram.tile([TSLICE, D], f32, name="rs_out")
                if do_collective:
                    nc.gpsimd.collective_compute(
                        "ReduceScatter",
                        Alu.add,
                        replica_groups=[list(range(NC))],
                        ins=[partial.opt()],
                        outs=[rs_out.opt()],
                    )
            rs_r = rs_out.rearrange("(t p) d -> t p d", p=128)

            # --- residual + LayerNorm on own 512-token slice ---
            with tc.tile_pool(name="ln", bufs=1) as ln_pool:
                for i in range(TSLICE // 128):
                    rs_sb = ln_pool.tile([128, D], f32, name="rs_sb", tag="rs")
                    nc.sync.dma_start(rs_sb[:], rs_r[i])
                    xr = ln_pool.tile([128, D], f32, name="xr", tag="xr")
                    nc.sync.dma_start(xr[:], xres_r[i])
                    nc.vector.tensor_tensor(rs_sb[:], rs_sb[:], xr[:], op=Alu.add)
                    mu = ln_pool.tile([128, 1], f32, name="mu", tag="mu")
                    nc.vector.reduce_sum(mu[:], rs_sb[:], axis=AX)
                    nc.vector.tensor_scalar_mul(mu[:], mu[:], 1.0 / D)
                    xc = ln_pool.tile([128, D], f32, name="xc", tag="xc")
                    nc.vector.tensor_scalar_sub(xc[:], rs_sb[:], mu[:])
                    sq = ln_pool.tile([128, D], f32, name="sq", tag="sq")
                    var = ln_pool.tile([128, 1], f32, name="var", tag="var")
                    nc.scalar.activation(sq[:], xc[:], Act.Square, accum_out=var[:])
                    std = ln_pool.tile([128, 1], f32, name="std", tag="std")
                    nc.scalar.activation(
                        std[:], var[:], Act.Sqrt, scale=1.0 / D, bias=eps_sb[:]
                    )
                    rstd = ln_pool.tile([128, 1], f32, name="rstd", tag="rstd")
                    nc.vector.reciprocal(rstd[:], std[:])
                    o_sb = ln_pool.tile([128, D], f32, name="o_sb", tag="o")
                    nc.vector.scalar_tensor_tensor(
                        o_sb[:], in0=xc[:], scalar=rstd[:], in1=gm_sb[:],
                        op0=Alu.mult, op1=Alu.mult,
                    )
                    nc.vector.tensor_tensor(o_sb[:], o_sb[:], bt_sb[:], op=Alu.add)
                    nc.sync.dma_start(out_r[i], o_sb[:])

    nc.finalize()
    return nc


CCAP = 1280              # per-expert token capacity (mean load is ~1024)
NTG = CCAP // 128        # 10 gathered token tiles
GBLOCKS = [512, 512, 256]
PAD_IDX = 2 ** 30        # sentinel row index; skipped via bounds_check


def _build_nc_sparse(do_scatter=True, do_collective=True, do_memset=True, debug_partial=False, unmerged=False):
    """Top-2-sparse expert FFN: each core computes only the tokens routed to
    its expert (gathered+padded to CCAP on host), scales by host-computed
    router weights, scatters rows into a zeroed dense [N, D] bf16 partial via
    indirect DMA, then ReduceScatter + residual + LayerNorm.

    All large inputs are host-pre-swizzled into SBUF layout so each loads
    with a single full-bandwidth DMA:
      xgsw  [128, ND*CCAP]  x^T gathered, per-block-major: blk | d0 | t
      w1sw  [128, ND*F]     W1[e]^T tiles column-concatenated by d0
      w2sw  [128, NF*D]     W2[e]^T tiles column-concatenated by f0
      cst1  [128, D+NF+NTG] b2 replicated | b1 cols | router-w cols
      cst2  [128, 2*D]      gamma replicated | beta replicated
    """
    import concourse.bacc as bacc
    import concourse.mybir as mybir
    import concourse.tile as tile
    from concourse.bass import IndirectOffsetOnAxis

    dt = mybir.dt
    f32, bf16, i32 = dt.float32, dt.bfloat16, dt.int32
    Alu = mybir.AluOpType
    Act = mybir.ActivationFunctionType
    AX = mybir.AxisListType.X

    nc = bacc.Bacc(num_devices=NC)

    xgsw = nc.dram_tensor("xgsw", [128, ND * CCAP], bf16, kind="ExternalInput")
    w1sw = nc.dram_tensor("w1sw", [128, ND * F], bf16, kind="ExternalInput")
    w2sw = nc.dram_tensor("w2sw", [128, NF * D], bf16, kind="ExternalInput")
    cst1 = nc.dram_tensor("cst1", [128, D + NF + NTG], f32, kind="ExternalInput")
    cst2 = nc.dram_tensor("cst2", [128, 2 * D], f32, kind="ExternalInput")
    scat = nc.dram_tensor("scat", [128, NTG], i32, kind="ExternalInput")
    xres = nc.dram_tensor("xres", [TSLICE, D], f32, kind="ExternalInput")
    out = nc.dram_tensor("out", [TSLICE, D], f32, kind="ExternalOutput")

    xres_r = xres.ap().rearrange("(a p) d -> a p d", p=128)
    out_r = out.ap().rearrange("(a p) d -> a p d", p=128)

    # per-block column offsets into xgsw: block b occupies ND*tb columns
    blk_off = [0]
    for tb in GBLOCKS:
        blk_off.append(blk_off[-1] + ND * tb)

    with tile.TileContext(nc) as tc:
        with (
            tc.tile_pool(name="wts", bufs=1) as wts,
            tc.tile_pool(name="xs", bufs=1) as xs_pool,
            tc.tile_pool(name="stg", bufs=2) as stg_pool,
            tc.tile_pool(name="stb", bufs=3) as stb_pool,
            tc.tile_pool(name="psh", bufs=2, space="PSUM") as psum_h,
            tc.tile_pool(name="pso", bufs=4, space="PSUM") as psum_o,
            tc.tile_pool(name="dram", bufs=1, space="DRAM") as dram,
        ):
            # first gathered block prefetch wins the DMA queue ahead of weights
            xb0 = xs_pool.tile([128, ND * 512], bf16, name="xb0", tag="xb")
            if unmerged:
                for d0 in range(ND):
                    nc.sync.dma_start(
                        xb0[:, d0 * GBLOCKS[0]:(d0 + 1) * GBLOCKS[0]],
                        xgsw[:, d0 * GBLOCKS[0]:(d0 + 1) * GBLOCKS[0]])
            else:
                nc.sync.dma_start(xb0[:, :ND * GBLOCKS[0]],
                                  xgsw[:, blk_off[0]:blk_off[1]])
            w1_all = wts.tile([128, ND * F], bf16, name="w1_all")
            if unmerged:
                for d0 in range(ND):
                    nc.sync.dma_start(w1_all[:, d0 * F:(d0 + 1) * F],
                                      w1sw[:, d0 * F:(d0 + 1) * F])
            else:
                _half = ND * F // 2
                nc.sync.dma_start(w1_all[:, :_half], w1sw[:, :_half])
                nc.sync.dma_start(w1_all[:, _half:], w1sw[:, _half:])
            c1_sb = wts.tile([128, D + NF + NTG], f32, name="c1_sb")
            nc.sync.dma_start(c1_sb[:], cst1[:])
            b2_sb = c1_sb[:, 0:D]
            b1_sb = c1_sb[:, D:D + NF]
            wg_sb = c1_sb[:, D + NF:D + NF + NTG]
            scat_sb = wts.tile([128, NTG], i32, name="scat_sb")
            nc.sync.dma_start(scat_sb[:], scat[:])
            eps_sb = wts.tile([128, 1], f32, name="eps_sb")
            nc.vector.memset(eps_sb[:], LN_EPS)
            zero_sb = wts.tile([128, 2 * D], bf16, name="zero_sb")
            nc.vector.memset(zero_sb[:], 0.0)

            partial = dram.tile([N, D], bf16, name="partial")
            partial_r = partial.rearrange("(t p) d -> t p d", p=128)
            partial_r2 = partial.rearrange("(t u p) d -> t u p d", u=2, p=128)
            if do_memset:
                for t in range(NT):
                    nc.sync.dma_start(partial_r[t], zero_sb[:, :D])

            w2_all = wts.tile([128, NF * D], bf16, name="w2_all")
            if unmerged:
                for f0 in range(NF):
                    nc.sync.dma_start(w2_all[:, f0 * D:(f0 + 1) * D],
                                      w2sw[:, f0 * D:(f0 + 1) * D])
            else:
                _half2 = NF * D // 2
                nc.sync.dma_start(w2_all[:, :_half2], w2sw[:, :_half2])
                nc.sync.dma_start(w2_all[:, _half2:], w2sw[:, _half2:])

            with tc.tile_pool(name="htp", bufs=1) as ht_pool:
                tok0 = 0
                for blk, tb in enumerate(GBLOCKS):
                    if blk == 0:
                        xb = xb0
                    else:
                        xb = xs_pool.tile([128, ND * 512], bf16, name="xb",
                                          tag="xb")
                        if unmerged:
                            for d0 in range(ND):
                                nc.sync.dma_start(
                                    xb[:, d0 * tb:(d0 + 1) * tb],
                                    xgsw[:, blk_off[blk] + d0 * tb:
                                         blk_off[blk] + (d0 + 1) * tb])
                        else:
                            nc.sync.dma_start(
                                xb[:, :ND * tb],
                                xgsw[:, blk_off[blk]:blk_off[blk + 1]])
                    ht = []
                    for f0 in range(NF):
                        hp = psum_h.tile([128, 512], f32, name="hp", tag="hp")
                        for d0 in range(ND):
                            nc.tensor.matmul(
                                hp[:, :tb],
                                lhsT=w1_all[:, d0 * F + f0 * 128:
                                            d0 * F + (f0 + 1) * 128],
                                rhs=xb[:, d0 * tb:(d0 + 1) * tb],
                                start=(d0 == 0),
                                stop=(d0 == ND - 1),
                            )
                        hs = ht_pool.tile([128, 512], bf16, name=f"ht{f0}",
                                          tag=f"ht{f0}")
                        nc.scalar.activation(
                            hs[:, :tb], hp[:, :tb], Act.Relu,
                            bias=b1_sb[:, f0:f0 + 1]
                        )
                        ht.append(hs)
                    for ts in range(tb // 128):
                        j = tok0 // 128 + ts
                        op0 = psum_o.tile([128, 512], f32, name="op0", tag="op")
                        op1 = psum_o.tile([128, 512], f32, name="op1", tag="op")
                        for f0 in range(NF):
                            nc.tensor.matmul(
                                op0[:],
                                lhsT=ht[f0][:, ts * 128:(ts + 1) * 128],
                                rhs=w2_all[:, f0 * D:f0 * D + 512],
                                start=(f0 == 0),
                                stop=(f0 == NF - 1),
                            )
                            nc.tensor.matmul(
                                op1[:],
                                lhsT=ht[f0][:, ts * 128:(ts + 1) * 128],
                                rhs=w2_all[:, f0 * D + 512:f0 * D + 1024],
                                start=(f0 == 0),
                                stop=(f0 == NF - 1),
                            )
                        stb = stb_pool.tile([128, D], bf16, name="stb", tag="stb")
                        for dn, op_ in enumerate((op0, op1)):
                            stf = stg_pool.tile([128, 512], f32, name="stf",
                                                tag="stf")
                            nc.vector.tensor_tensor(
                                stf[:], op_[:], b2_sb[:, dn * 512:(dn + 1) * 512],
                                op=Alu.add,
                            )
                            nc.vector.tensor_scalar_mul(
                                stb[:, dn * 512:(dn + 1) * 512], stf[:],
                                wg_sb[:, j:j + 1]
                            )
                        if do_scatter:
                            nc.gpsimd.indirect_dma_start(
                                out=partial[:],
                                out_offset=IndirectOffsetOnAxis(
                                    ap=scat_sb[:, j:j + 1], axis=0
                                ),
                                in_=stb[:],
                                in_offset=None,
                                bounds_check=N - 1,
                                oob_is_err=False,
                            )
                        else:
                            nc.sync.dma_start(partial_r[j], stb[:])
                    tok0 += tb

                rs_out = dram.tile([TSLICE, D], bf16, name="rs_out")
                if do_collective:
                    nc.gpsimd.collective_compute(
                        "ReduceScatter",
                        Alu.add,
                        replica_groups=[list(range(NC))],
                        ins=[partial.opt()],
                        outs=[rs_out.opt()],
                    )
            rs_r = rs_out.rearrange("(t p) d -> t p d", p=128)

            if debug_partial:
                with tc.tile_pool(name="dbg", bufs=2) as dbg_pool:
                    for i in range(TSLICE // 128):
                        dsb = dbg_pool.tile([128, D], bf16, name="dsb", tag="d")
                        nc.sync.dma_start(dsb[:], partial_r[i])
                        dsf = dbg_pool.tile([128, D], f32, name="dsf", tag="df")
                        nc.vector.tensor_copy(dsf[:], dsb[:])
                        nc.sync.dma_start(out_r[i], dsf[:])
            with tc.tile_pool(name="ln", bufs=1) as ln_pool:
                if debug_partial:
                    ln_pool  # placeholder; LN skipped in debug mode
                if debug_partial:
                    c2_sb = None
                else:
                    c2_sb = ln_pool.tile([128, 2 * D], f32, name="c2_sb", tag="c2")
                    nc.sync.dma_start(c2_sb[:], cst2[:])
                gm_sb = None if debug_partial else c2_sb[:, 0:D]
                bt_sb = None if debug_partial else c2_sb[:, D:2 * D]
                for i in range(0 if debug_partial else TSLICE // 128):
                    rs_sb = ln_pool.tile([128, D], bf16, name="rs_sb", tag="rs")
                    nc.sync.dma_start(rs_sb[:], rs_r[i])
                    xr = ln_pool.tile([128, D], f32, name="xr", tag="xr")
                    nc.sync.dma_start(xr[:], xres_r[i])
                    y = ln_pool.tile([128, D], f32, name="y", tag="y")
                    nc.vector.tensor_tensor(y[:], rs_sb[:], xr[:], op=Alu.add)
                    mu = ln_pool.tile([128, 1], f32, name="mu", tag="mu")
                    nc.vector.reduce_sum(mu[:], y[:], axis=AX)
                    nc.vector.tensor_scalar_mul(mu[:], mu[:], 1.0 / D)
                    xc = ln_pool.tile([128, D], f32, name="xc", tag="xc")
                    nc.vector.tensor_scalar_sub(xc[:], y[:], mu[:])
                    sq = ln_pool.tile([128, D], f32, name="sq", tag="sq")
                    var = ln_pool.tile([128, 1], f32, name="var", tag="var")
                    nc.scalar.activation(sq[:], xc[:], Act.Square,
                                         accum_out=var[:])
                    std = ln_pool.tile([128, 1], f32, name="std", tag="std")
                    nc.scalar.activation(std[:], var[:], Act.Sqrt,
                                         scale=1.0 / D, bias=eps_sb[:])
                    rstd = ln_pool.tile([128, 1], f32, name="rstd", tag="rstd")
                    nc.vector.reciprocal(rstd[:], std[:])
                    o_sb = ln_pool.tile([128, D], f32, name="o_sb", tag="o")
                    nc.vector.scalar_tensor_tensor(
                        o_sb[:], in0=xc[:], scalar=rstd[:], in1=gm_sb,
                        op0=Alu.mult, op1=Alu.mult,
                    )
                    nc.vector.tensor_tensor(o_sb[:], o_sb[:], bt_sb,
                                            op=Alu.add)
                    nc.sync.dma_start(out_r[i], o_sb[:])

    nc.finalize()
    return nc


C2 = 176                # capacity per (expert, owner) segment (mean ~128)
C3 = C2 * NC            # 1408 gathered tokens per expert
NTG3 = C3 // 128        # 11 gathered tiles
C2A, C2B = 96, 80       # A2A split: first/last rows of each segment
SPLA, SPLB = C2A * NC, C2B * NC      # 768 / 640 rows
NTA, NTB = SPLA // 128, SPLB // 128  # 6 / 5 tiles
GBLOCKS3 = [512, 256, 512, 128]      # blocks 0-1 fill A, 2-3 fill B

# --- sparse4: balanced token->owner assignment, tighter capacity ---
C4 = 144                 # capacity per (expert, owner) segment
C34 = C4 * NC            # 1152 gathered tokens per expert
NTG4 = C34 // 128        # 9 gathered tiles
CH4 = (64, 48, 32)       # per-segment A2A chunk sizes (3 collectives)
SPL4 = tuple(c * NC for c in CH4)          # 512 / 384 / 256 rows
NT4 = tuple(s // 128 for s in SPL4)        # 4 / 3 / 2 tiles
GBLOCKS4 = list(SPL4)    # matmul blocks == chunk fills


def _build_nc_sparse4():
    """v4: like sparse3 but with host-balanced token ownership so every
    (expert, owner) segment fits in C4=144 rows (C3 1408 -> 1152, 18% less
    matmul work), W1 in f0-major layout loaded as 8 small tiles so the first
    matmul group starts after ~2 MB of DMA instead of 9 MB, and the AllToAll
    split in 3 chunks issued as soon as each block's outputs are written so
    only the last small chunk (256 rows) is exposed at the tail.
    """
    import concourse.bacc as bacc
    import concourse.mybir as mybir
    import concourse.tile as tile

    dt = mybir.dt
    f32, bf16, i32 = dt.float32, dt.bfloat16, dt.int32
    Alu = mybir.AluOpType
    Act = mybir.ActivationFunctionType
    AX = mybir.AxisListType.X

    nc = bacc.Bacc(num_devices=NC)

    xgsw = nc.dram_tensor("xgsw", [128, ND * C34], bf16, kind="ExternalInput")
    w1sw = nc.dram_tensor("w1sw", [128, ND * F], bf16, kind="ExternalInput")
    w2sw = nc.dram_tensor("w2sw", [128, NF * D], bf16, kind="ExternalInput")
    cst1 = nc.dram_tensor("cst1", [128, D + NF + NTG4], f32, kind="ExternalInput")
    # cst2: gamma_rep | beta_rep | iota row 0..511
    cst2 = nc.dram_tensor("cst2", [128, 2 * D + 512], f32, kind="ExternalInput")
    locs = nc.dram_tensor("locs", [128, NTG4], i32, kind="ExternalInput")
    xres = nc.dram_tensor("xres", [TSLICE, D], f32, kind="ExternalInput")
    out = nc.dram_tensor("out", [TSLICE, D], f32, kind="ExternalOutput")

    xres_r = xres.ap().rearrange("(a p) d -> a p d", p=128)
    out_r = out.ap().rearrange("(a p) d -> a p d", p=128)

    blk_off = [0]
    for tb in GBLOCKS4:
        blk_off.append(blk_off[-1] + ND * tb)

    W1G = 8               # w1 loaded as 8 tiles of 4 f-tiles each
    W1C = NF // W1G * ND * 128          # columns per w1 tile (4096)
    W2G = 4               # w2 loaded as 4 tiles of 8 f-tiles each
    W2C = NF // W2G * D                 # columns per w2 tile (8192)

    with tile.TileContext(nc) as tc:
        with (
            tc.tile_pool(name="wts", bufs=1) as wts,
            tc.tile_pool(name="xs", bufs=1) as xs_pool,
            tc.tile_pool(name="stg", bufs=2) as stg_pool,
            tc.tile_pool(name="stb", bufs=3) as stb_pool,
            tc.tile_pool(name="psh", bufs=2, space="PSUM") as psum_h,
            tc.tile_pool(name="pso", bufs=6, space="PSUM") as psum_o,
            tc.tile_pool(name="dram", bufs=1, space="DRAM") as dram,
        ):
            # DMA order = need order: x block 0, first w1 tile, biases, rest
            # of w1, w2, x blocks 1-2.
            xb0 = xs_pool.tile([128, ND * GBLOCKS4[0]], bf16, name="xb0",
                               tag="xb0")
            nc.sync.dma_start(xb0[:], xgsw[:, blk_off[0]:blk_off[1]])
            w1g = []
            for g in range(W1G):
                t = wts.tile([128, W1C], bf16, name=f"w1g{g}", tag=f"w1g{g}")
                nc.sync.dma_start(t[:], w1sw[:, g * W1C:(g + 1) * W1C])
                w1g.append(t)
                if g == 0:
                    c1_sb = wts.tile([128, D + NF + NTG4], f32, name="c1_sb")
                    nc.sync.dma_start(c1_sb[:], cst1[:])
            b2_sb = c1_sb[:, 0:D]
            b1_sb = c1_sb[:, D:D + NF]
            wg_sb = c1_sb[:, D + NF:D + NF + NTG4]
            locs_sb = wts.tile([128, NTG4], i32, name="locs_sb")
            nc.sync.dma_start(locs_sb[:], locs[:])
            eps_sb = wts.tile([128, 1], f32, name="eps_sb")
            nc.vector.memset(eps_sb[:], LN_EPS)
            w2g = []
            for g in range(W2G):
                t = wts.tile([128, W2C], bf16, name=f"w2g{g}", tag=f"w2g{g}")
                nc.sync.dma_start(t[:], w2sw[:, g * W2C:(g + 1) * W2C])
                w2g.append(t)
            xbs = [xb0]
            for blk in (1, 2):
                t = xs_pool.tile([128, ND * GBLOCKS4[blk]], bf16,
                                 name=f"xb{blk}", tag=f"xb{blk}")
                nc.sync.dma_start(t[:], xgsw[:, blk_off[blk]:blk_off[blk + 1]])
                xbs.append(t)

            a2a_in, a2a_out, a2a_in_r = [], [], []
            for ci, spl in enumerate(SPL4):
                ti = dram.tile([spl, D], bf16, name=f"a2a_in{ci}")
                to = dram.tile([spl, D], bf16, name=f"a2a_out{ci}")
                a2a_in.append(ti)
                a2a_out.append(to)
                a2a_in_r.append(ti.rearrange("(t p) d -> t p d", p=128))

            with tc.tile_pool(name="htp", bufs=1) as ht_pool:
                tok0 = 0
                for blk, tb in enumerate(GBLOCKS4):
                    xb = xbs[blk]
                    ht = []
                    for f0 in range(NF):
                        hp = psum_h.tile([128, 512], f32, name="hp", tag="hp")
                        w1t = w1g[f0 // 4]
                        fo = (f0 % 4) * ND * 128
                        for d0 in range(ND):
                            nc.tensor.matmul(
                                hp[:, :tb],
                                lhsT=w1t[:, fo + d0 * 128:fo + (d0 + 1) * 128],
                                rhs=xb[:, d0 * tb:(d0 + 1) * tb],
                                start=(d0 == 0),
                                stop=(d0 == ND - 1),
                            )
                        hs = ht_pool.tile([128, 512], bf16, name=f"ht{f0}",
                                          tag=f"ht{f0}")
                        nc.scalar.activation(
                            hs[:, :tb], hp[:, :tb], Act.Relu,
                            bias=b1_sb[:, f0:f0 + 1]
                        )
                        ht.append(hs)
                    for ts in range(tb // 128):
                        j = tok0 // 128 + ts
                        op0 = psum_o.tile([128, 512], f32, name="op0", tag="op")
                        op1 = psum_o.tile([128, 512], f32, name="op1", tag="op")
                        for f0 in range(NF):
                            w2t = w2g[f0 // 8]
                            fo = (f0 % 8) * D
                            nc.tensor.matmul(
                                op0[:],
                                lhsT=ht[f0][:, ts * 128:(ts + 1) * 128],
                                rhs=w2t[:, fo:fo + 512],
                                start=(f0 == 0),
                                stop=(f0 == NF - 1),
                            )
                            nc.tensor.matmul(
                                op1[:],
                                lhsT=ht[f0][:, ts * 128:(ts + 1) * 128],
                                rhs=w2t[:, fo + 512:fo + 1024],
                                start=(f0 == 0),
                                stop=(f0 == NF - 1),
                            )
                        stb = stb_pool.tile([128, D], bf16, name="stb", tag="stb")
                        for dn, op_ in enumerate((op0, op1)):
                            stf = stg_pool.tile([128, 512], f32, name="stf",
                                                tag="stf")
                            nc.vector.tensor_tensor(
                                stf[:], op_[:], b2_sb[:, dn * 512:(dn + 1) * 512],
                                op=Alu.add,
                            )
                            nc.vector.tensor_scalar_mul(
                                stb[:, dn * 512:(dn + 1) * 512], stf[:],
                                wg_sb[:, j:j + 1]
                            )
                        nc.sync.dma_start(a2a_in_r[blk][ts], stb[:])
                    tok0 += tb
                    nc.gpsimd.collective_compute(
                        "AllToAll",
                        Alu.bypass,
                        replica_groups=[list(range(NC))],
                        ins=[a2a_in[blk].opt()],
                        outs=[a2a_out[blk].opt()],
                    )
            a2a_out_r = [t.rearrange("(t p) d -> t p d", p=128)
                         for t in a2a_out]
            # kt tile -> (chunk, tile within chunk)
            kt_src = []
            for ci, nt in enumerate(NT4):
                for t in range(nt):
                    kt_src.append((ci, t))

            # --- combine received segments into own slice + residual + LN ---
            with tc.tile_pool(name="cmb", bufs=1) as cmb_pool:
                c2_sb = cmb_pool.tile([128, 2 * D + 512], f32, name="c2_sb",
                                      tag="c2")
                nc.sync.dma_start(c2_sb[:], cst2[:])
                gm_sb = c2_sb[:, 0:D]
                bt_sb = c2_sb[:, D:2 * D]
                iota_sb = c2_sb[:, 2 * D:2 * D + 512]
                liv = cmb_pool.tile([128, NTG4], f32, name="liv", tag="liv")
                nc.vector.tensor_copy(liv[:], locs_sb[:])
                # 8 PSUM banks accumulate all (mt, dn) outputs across kt
                pst = []
                for mt in range(TSLICE // 128):
                    pa = psum_o.tile([128, 512], f32, name=f"cA{mt}", tag="op")
                    pb = (psum_h if mt >= 2 else psum_o).tile(
                        [128, 512], f32, name=f"cB{mt}",
                        tag="hp" if mt >= 2 else "op")
                    pst.append((pa, pb))
                for kt in range(NTG4):
                    ci, t = kt_src[kt]
                    s = cmb_pool.tile([128, 512], bf16, name="sel", tag="sel",
                                      bufs=2)
                    nc.vector.tensor_scalar(
                        s[:], iota_sb, liv[:, kt:kt + 1], None,
                        op0=Alu.is_equal,
                    )
                    rt = cmb_pool.tile([128, D], bf16, name="arow", tag="arow",
                                       bufs=2)
                    nc.sync.dma_start(rt[:], a2a_out_r[ci][t])
                    for mt in range(TSLICE // 128):
                        nc.tensor.matmul(
                            pst[mt][0][:],
                            lhsT=s[:, mt * 128:(mt + 1) * 128],
                            rhs=rt[:, 0:512],
                            start=(kt == 0),
                            stop=(kt == NTG4 - 1),
                        )
                        nc.tensor.matmul(
                            pst[mt][1][:],
                            lhsT=s[:, mt * 128:(mt + 1) * 128],
                            rhs=rt[:, 512:1024],
                            start=(kt == 0),
                            stop=(kt == NTG4 - 1),
                        )
                for mt in range(TSLICE // 128):
                    psA, psB = pst[mt]
                    xr = cmb_pool.tile([128, D], f32, name="xr", tag="xr")
                    nc.sync.dma_start(xr[:], xres_r[mt])
                    y = cmb_pool.tile([128, D], f32, name="y", tag="y")
                    nc.vector.tensor_tensor(y[:, 0:512], psA[:], xr[:, 0:512],
                                            op=Alu.add)
                    nc.vector.tensor_tensor(y[:, 512:1024], psB[:],
                                            xr[:, 512:1024], op=Alu.add)
                    mu = cmb_pool.tile([128, 1], f32, name="mu", tag="mu")
                    nc.vector.reduce_sum(mu[:], y[:], axis=AX)
                    nc.vector.tensor_scalar_mul(mu[:], mu[:], 1.0 / D)
                    xc = cmb_pool.tile([128, D], f32, name="xc", tag="xc")
                    nc.vector.tensor_scalar_sub(xc[:], y[:], mu[:])
                    sq = cmb_pool.tile([128, D], f32, name="sq", tag="sq")
                    var = cmb_pool.tile([128, 1], f32, name="var", tag="var")
                    nc.scalar.activation(sq[:], xc[:], Act.Square,
                                         accum_out=var[:])
                    std = cmb_pool.tile([128, 1], f32, name="std", tag="std")
                    nc.scalar.activation(std[:], var[:], Act.Sqrt,
                                         scale=1.0 / D, bias=eps_sb[:])
                    rstd = cmb_pool.tile([128, 1], f32, name="rstd", tag="rstd")
                    nc.vector.reciprocal(rstd[:], std[:])
                    o_sb = cmb_pool.tile([128, D], f32, name="o_sb", tag="o")
                    nc.vector.scalar_tensor_tensor(
                        o_sb[:], in0=xc[:], scalar=rstd[:], in1=gm_sb,
                        op0=Alu.mult, op1=Alu.mult,
                    )
                    nc.vector.tensor_tensor(o_sb[:], o_sb[:], bt_sb,
                                            op=Alu.add)
                    nc.sync.dma_start(out_r[mt], o_sb[:])

    nc.finalize()
    return nc


def _balance_owners(i1, i2):
    """Assign each token an owner core (512 tokens per core) keeping every
    (expert, owner) segment <= C4. Greedy: hardest tokens first, pick the
    core minimizing the resulting max of the token's two expert segments.
    Returns owner[N] or None if the greedy result exceeds capacity."""
    esz = np.bincount(np.concatenate([i1, i2]), minlength=E)
    order = np.argsort(-(esz[i1] + esz[i2]), kind="stable")
    cnt = np.zeros((E, NC), np.int32)
    load = np.zeros(NC, np.int32)
    owner = np.full(N, -1, np.int32)
    for t in order:
        e1, e2 = i1[t], i2[t]
        best, bkey = -1, None
        for r in range(NC):
            if load[r] >= TSLICE:
                continue
            key = (max(cnt[e1, r], cnt[e2, r]), cnt[e1, r] + cnt[e2, r],
                   load[r])
            if bkey is None or key < bkey:
                bkey, best = key, r
        owner[t] = best
        cnt[e1, best] += 1
        cnt[e2, best] += 1
        load[best] += 1
    if cnt.max() > C4:
        return None
    return owner


def _build_in_maps_sparse4(tgt, Wr, br, W1, b1, W2, b2, gamma, beta):
    """Host prep for sparse4. Returns (in_maps, tok_lists) or None if the
    balancer cannot fit every segment in C4."""
    f32 = np.float32
    x = np.ascontiguousarray(np.asarray(tgt, f32).reshape(N, D))
    Wr = np.asarray(Wr, f32)
    br = np.asarray(br, f32)
    W1 = np.asarray(W1, f32)
    b1 = np.asarray(b1, f32)
    W2 = np.asarray(W2, f32)
    b2 = np.asarray(b2, f32)
    gamma = np.asarray(gamma, f32)
    beta = np.asarray(beta, f32)

    i1, i2, w1, w2 = _route_host(x, Wr, br)
    owner = _balance_owners(i1, i2)
    if owner is None:
        return None
    tok_lists = [np.nonzero(owner == r)[0] for r in range(NC)]
    pos_in_owner = np.empty(N, np.int64)
    for r in range(NC):
        pos_in_owner[tok_lists[r]] = np.arange(TSLICE)

    iota = np.broadcast_to(np.arange(512, dtype=f32), (128, 512))
    cst2 = np.ascontiguousarray(np.concatenate([
        np.broadcast_to(gamma, (128, D)),
        np.broadcast_to(beta, (128, D)),
        iota,
    ], axis=1))

    xt = x.T  # [D, N] view

    # chunk slot offsets within a segment and row offsets within the
    # gathered column order
    ch_base = [0]
    for c in CH4:
        ch_base.append(ch_base[-1] + c)
    row_base = [0]
    for s in SPL4:
        row_base.append(row_base[-1] + s)

    all_idx = []      # gathered global token per slot (per expert)
    all_w = []        # router weight per slot
    all_loc = []      # owner-local index per slot
    for e in range(NC):
        sel = (i1 == e) | (i2 == e)
        idx_e = np.nonzero(sel)[0]
        w_e = np.where(i1[idx_e] == e, w1[idx_e], w2[idx_e]).astype(f32)
        seg_idx = np.zeros(C34, np.int64)
        seg_w = np.zeros(C34, f32)
        seg_loc = np.full(C34, PAD_IDX, np.int64)
        for r in range(NC):
            m = owner[idx_e] == r
            li = idx_e[m]
            wi = w_e[m]
            if li.size > C4:
                return None
            o = 0
            for ci, csz in enumerate(CH4):
                nhere = min(max(li.size - o, 0), csz)
                if nhere > 0:
                    b0 = row_base[ci] + r * csz
                    seg_idx[b0:b0 + nhere] = li[o:o + nhere]
                    seg_w[b0:b0 + nhere] = wi[o:o + nhere]
                    seg_loc[b0:b0 + nhere] = pos_in_owner[li[o:o + nhere]]
                o += nhere
        all_idx.append(seg_idx)
        all_w.append(seg_w)
        all_loc.append(seg_loc)

    in_maps = []
    for e in range(NC):
        seg_idx, seg_w = all_idx[e], all_w[e]
        xg = xt[:, seg_idx].reshape(ND, 128, C34)
        parts = []
        c0 = 0
        for tb in GBLOCKS4:
            parts.append(xg[:, :, c0:c0 + tb].transpose(1, 0, 2)
                         .reshape(128, ND * tb))
            c0 += tb
        xgsw = np.ascontiguousarray(np.concatenate(parts, axis=1)).astype(BF16)
        # f0-major W1: [p, f0, d0, j]
        w1sw = np.ascontiguousarray(
            W1[e].T.reshape(ND, 128, NF, 128).transpose(1, 2, 0, 3)
            .reshape(128, NF * ND * 128)
        ).astype(BF16)
        w2sw = np.ascontiguousarray(
            W2[e].T.reshape(NF, 128, D).transpose(1, 0, 2).reshape(128, NF * D)
        ).astype(BF16)
        cst1 = np.ascontiguousarray(np.concatenate([
            np.broadcast_to(b2[e], (128, D)),
            b1[e].reshape(NF, 128).T,
            seg_w.reshape(NTG4, 128).T,
        ], axis=1))
        # receiver-side locs: core e's a2a_out chunk ci rows [s*csz,(s+1)*csz)
        # hold expert s's tokens owned by e
        loc_rows = np.full(C34, PAD_IDX, np.int64)
        for s in range(NC):
            for ci, csz in enumerate(CH4):
                src0 = row_base[ci] + e * csz
                dst0 = row_base[ci] + s * csz
                loc_rows[dst0:dst0 + csz] = all_loc[s][src0:src0 + csz]
        locs = np.ascontiguousarray(
            loc_rows.astype(np.int32).reshape(NTG4, 128).T)
        in_maps.append({
            "xgsw": xgsw,
            "w1sw": w1sw,
            "w2sw": w2sw,
            "cst1": cst1,
            "cst2": cst2,
            "locs": locs,
            "xres": np.ascontiguousarray(x[tok_lists[e]]),
        })
    return in_maps, tok_lists


def _build_nc_sparse3():
    """v3: top-2 sparse with AllToAll combine.

    Gathered tokens for expert e are laid out as 8 owner segments of C2
    rows (tokens owned by core r, padded). Each core computes its expert's
    weighted outputs into a2a_in [C3, D] bf16 with plain DMAs, AllToAll
    swaps owner segments, and each core then sums the 8 received segments
    into its own 512-token slice with a one-hot matmul on PE, fused with
    residual + LayerNorm.
    """
    import concourse.bacc as bacc
    import concourse.mybir as mybir
    import concourse.tile as tile

    dt = mybir.dt
    f32, bf16, i32 = dt.float32, dt.bfloat16, dt.int32
    Alu = mybir.AluOpType
    Act = mybir.ActivationFunctionType
    AX = mybir.AxisListType.X

    nc = bacc.Bacc(num_devices=NC)

    xgsw = nc.dram_tensor("xgsw", [128, ND * C3], bf16, kind="ExternalInput")
    w1sw = nc.dram_tensor("w1sw", [128, ND * F], bf16, kind="ExternalInput")
    w2sw = nc.dram_tensor("w2sw", [128, NF * D], bf16, kind="ExternalInput")
    cst1 = nc.dram_tensor("cst1", [128, D + NF + NTG3], f32, kind="ExternalInput")
    # cst2: gamma_rep | beta_rep | iota row 0..511
    cst2 = nc.dram_tensor("cst2", [128, 2 * D + 512], f32, kind="ExternalInput")
    locs = nc.dram_tensor("locs", [128, NTG3], i32, kind="ExternalInput")
    xres = nc.dram_tensor("xres", [TSLICE, D], f32, kind="ExternalInput")
    out = nc.dram_tensor("out", [TSLICE, D], f32, kind="ExternalOutput")

    xres_r = xres.ap().rearrange("(a p) d -> a p d", p=128)
    out_r = out.ap().rearrange("(a p) d -> a p d", p=128)

    blk_off = [0]
    for tb in GBLOCKS3:
        blk_off.append(blk_off[-1] + ND * tb)

    with tile.TileContext(nc) as tc:
        with (
            tc.tile_pool(name="wts", bufs=1) as wts,
            tc.tile_pool(name="xs", bufs=1) as xs_pool,
            tc.tile_pool(name="stg", bufs=2) as stg_pool,
            tc.tile_pool(name="stb", bufs=3) as stb_pool,
            tc.tile_pool(name="psh", bufs=2, space="PSUM") as psum_h,
            tc.tile_pool(name="pso", bufs=6, space="PSUM") as psum_o,
            tc.tile_pool(name="dram", bufs=1, space="DRAM") as dram,
        ):
            xb0 = xs_pool.tile([128, ND * 512], bf16, name="xb0", tag="xb")
            nc.sync.dma_start(xb0[:, :ND * GBLOCKS3[0]],
                              xgsw[:, blk_off[0]:blk_off[1]])
            w1_all = wts.tile([128, ND * F], bf16, name="w1_all")
            _half = ND * F // 2
            nc.sync.dma_start(w1_all[:, :_half], w1sw[:, :_half])
            nc.sync.dma_start(w1_all[:, _half:], w1sw[:, _half:])
            c1_sb = wts.tile([128, D + NF + NTG3], f32, name="c1_sb")
            nc.sync.dma_start(c1_sb[:], cst1[:])
            b2_sb = c1_sb[:, 0:D]
            b1_sb = c1_sb[:, D:D + NF]
            wg_sb = c1_sb[:, D + NF:D + NF + NTG3]
            locs_sb = wts.tile([128, NTG3], i32, name="locs_sb")
            nc.sync.dma_start(locs_sb[:], locs[:])
            eps_sb = wts.tile([128, 1], f32, name="eps_sb")
            nc.vector.memset(eps_sb[:], LN_EPS)

            a2a_inA = dram.tile([SPLA, D], bf16, name="a2a_inA")
            a2a_inA_r = a2a_inA.rearrange("(t p) d -> t p d", p=128)
            a2a_inB = dram.tile([SPLB, D], bf16, name="a2a_inB")
            a2a_inB_r = a2a_inB.rearrange("(t p) d -> t p d", p=128)
            a2a_outA = dram.tile([SPLA, D], bf16, name="a2a_outA")
            a2a_outB = dram.tile([SPLB, D], bf16, name="a2a_outB")

            w2_all = wts.tile([128, NF * D], bf16, name="w2_all")
            _half2 = NF * D // 2
            nc.sync.dma_start(w2_all[:, :_half2], w2sw[:, :_half2])
            nc.sync.dma_start(w2_all[:, _half2:], w2sw[:, _half2:])

            with tc.tile_pool(name="htp", bufs=1) as ht_pool:
                tok0 = 0
                for blk, tb in enumerate(GBLOCKS3):
                    if blk == 0:
                        xb = xb0
                    else:
                        xb = xs_pool.tile([128, ND * 512], bf16, name="xb",
                                          tag="xb")
                        nc.sync.dma_start(
                            xb[:, :ND * tb],
                            xgsw[:, blk_off[blk]:blk_off[blk + 1]])
                    ht = []
                    for f0 in range(NF):
                        hp = psum_h.tile([128, 512], f32, name="hp", tag="hp")
                        for d0 in range(ND):
                            nc.tensor.matmul(
                                hp[:, :tb],
                                lhsT=w1_all[:, d0 * F + f0 * 128:
                                            d0 * F + (f0 + 1) * 128],
                                rhs=xb[:, d0 * tb:(d0 + 1) * tb],
                                start=(d0 == 0),
                                stop=(d0 == ND - 1),
                            )
                        hs = ht_pool.tile([128, 512], bf16, name=f"ht{f0}",
                                          tag=f"ht{f0}")
                        nc.scalar.activation(
                            hs[:, :tb], hp[:, :tb], Act.Relu,
                            bias=b1_sb[:, f0:f0 + 1]
                        )
                        ht.append(hs)
                    for ts in range(tb // 128):
                        j = tok0 // 128 + ts
                        op0 = psum_o.tile([128, 512], f32, name="op0", tag="op")
                        op1 = psum_o.tile([128, 512], f32, name="op1", tag="op")
                        for f0 in range(NF):
                            nc.tensor.matmul(
                                op0[:],
                                lhsT=ht[f0][:, ts * 128:(ts + 1) * 128],
                                rhs=w2_all[:, f0 * D:f0 * D + 512],
                                start=(f0 == 0),
                                stop=(f0 == NF - 1),
                            )
                            nc.tensor.matmul(
                                op1[:],
                                lhsT=ht[f0][:, ts * 128:(ts + 1) * 128],
                                rhs=w2_all[:, f0 * D + 512:f0 * D + 1024],
                                start=(f0 == 0),
                                stop=(f0 == NF - 1),
                            )
                        stb = stb_pool.tile([128, D], bf16, name="stb", tag="stb")
                        for dn, op_ in enumerate((op0, op1)):
                            stf = stg_pool.tile([128, 512], f32, name="stf",
                                                tag="stf")
                            nc.vector.tensor_tensor(
                                stf[:], op_[:], b2_sb[:, dn * 512:(dn + 1) * 512],
                                op=Alu.add,
                            )
                            nc.vector.tensor_scalar_mul(
                                stb[:, dn * 512:(dn + 1) * 512], stf[:],
                                wg_sb[:, j:j + 1]
                            )
                        if j < NTA:
                            nc.sync.dma_start(a2a_inA_r[j], stb[:])
                        else:
                            nc.sync.dma_start(a2a_inB_r[j - NTA], stb[:])
                    tok0 += tb
                    if blk == 1:
                        nc.gpsimd.collective_compute(
                            "AllToAll",
                            Alu.bypass,
                            replica_groups=[list(range(NC))],
                            ins=[a2a_inA.opt()],
                            outs=[a2a_outA.opt()],
                        )
                nc.gpsimd.collective_compute(
                    "AllToAll",
                    Alu.bypass,
                    replica_groups=[list(range(NC))],
                    ins=[a2a_inB.opt()],
                    outs=[a2a_outB.opt()],
                )
            a2a_outA_r = a2a_outA.rearrange("(t p) d -> t p d", p=128)
            a2a_outB_r = a2a_outB.rearrange("(t p) d -> t p d", p=128)

            # --- combine received segments into own slice + residual + LN ---
            with tc.tile_pool(name="cmb", bufs=1) as cmb_pool:
                c2_sb = cmb_pool.tile([128, 2 * D + 512], f32, name="c2_sb",
                                      tag="c2")
                nc.sync.dma_start(c2_sb[:], cst2[:])
                gm_sb = c2_sb[:, 0:D]
                bt_sb = c2_sb[:, D:2 * D]
                iota_sb = c2_sb[:, 2 * D:2 * D + 512]
                liv = cmb_pool.tile([128, NTG3], f32, name="liv", tag="liv")
                nc.vector.tensor_copy(liv[:], locs_sb[:])
                # 8 PSUM banks accumulate all (mt, dn) outputs across kt
                pst = []
                for mt in range(TSLICE // 128):
                    pa = psum_o.tile([128, 512], f32, name=f"cA{mt}", tag="op")
                    pb = (psum_h if mt >= 2 else psum_o).tile(
                        [128, 512], f32, name=f"cB{mt}",
                        tag="hp" if mt >= 2 else "op")
                    pst.append((pa, pb))
                for kt in range(NTG3):
                    s = cmb_pool.tile([128, 512], bf16, name="sel", tag="sel",
                                      bufs=2)
                    nc.vector.tensor_scalar(
                        s[:], iota_sb, liv[:, kt:kt + 1], None,
                        op0=Alu.is_equal,
                    )
                    rt = cmb_pool.tile([128, D], bf16, name="arow", tag="arow",
                                       bufs=2)
                    if kt < NTA:
                        nc.sync.dma_start(rt[:], a2a_outA_r[kt])
                    else:
                        nc.sync.dma_start(rt[:], a2a_outB_r[kt - NTA])
                    for mt in range(TSLICE // 128):
                        nc.tensor.matmul(
                            pst[mt][0][:],
                            lhsT=s[:, mt * 128:(mt + 1) * 128],
                            rhs=rt[:, 0:512],
                            start=(kt == 0),
                            stop=(kt == NTG3 - 1),
                        )
                        nc.tensor.matmul(
                            pst[mt][1][:],
                            lhsT=s[:, mt * 128:(mt + 1) * 128],
                            rhs=rt[:, 512:1024],
                            start=(kt == 0),
                            stop=(kt == NTG3 - 1),
                        )
                for mt in range(TSLICE // 128):
                    psA, psB = pst[mt]
                    xr = cmb_pool.tile([128, D], f32, name="xr", tag="xr")
                    nc.sync.dma_start(xr[:], xres_r[mt])
                    y = cmb_pool.tile([128, D], f32, name="y", tag="y")
                    nc.vector.tensor_tensor(y[:, 0:512], psA[:], xr[:, 0:512],
                                            op=Alu.add)
                    nc.vector.tensor_tensor(y[:, 512:1024], psB[:],
                                            xr[:, 512:1024], op=Alu.add)
                    mu = cmb_pool.tile([128, 1], f32, name="mu", tag="mu")
                    nc.vector.reduce_sum(mu[:], y[:], axis=AX)
                    nc.vector.tensor_scalar_mul(mu[:], mu[:], 1.0 / D)
                    xc = cmb_pool.tile([128, D], f32, name="xc", tag="xc")
                    nc.vector.tensor_scalar_sub(xc[:], y[:], mu[:])
                    sq = cmb_pool.tile([128, D], f32, name="sq", tag="sq")
                    var = cmb_pool.tile([128, 1], f32, name="var", tag="var")
                    nc.scalar.activation(sq[:], xc[:], Act.Square,
                                         accum_out=var[:])
                    std = cmb_pool.tile([128, 1], f32, name="std", tag="std")
                    nc.scalar.activation(std[:], var[:], Act.Sqrt,
                                         scale=1.0 / D, bias=eps_sb[:])
                    rstd = cmb_pool.tile([128, 1], f32, name="rstd", tag="rstd")
                    nc.vector.reciprocal(rstd[:], std[:])
                    o_sb = cmb_pool.tile([128, D], f32, name="o_sb", tag="o")
                    nc.vector.scalar_tensor_tensor(
                        o_sb[:], in0=xc[:], scalar=rstd[:], in1=gm_sb,
                        op0=Alu.mult, op1=Alu.mult,
                    )
                    nc.vector.tensor_tensor(o_sb[:], o_sb[:], bt_sb,
                                            op=Alu.add)
                    nc.sync.dma_start(out_r[mt], o_sb[:])

    nc.finalize()
    return nc


def _build_in_maps_sparse3(tgt, Wr, br, W1, b1, W2, b2, gamma, beta):
    """Segment-padded gather for the AllToAll combine. Returns None when any
    (expert, owner) segment exceeds C2 (caller falls back)."""
    f32 = np.float32
    x = np.ascontiguousarray(np.asarray(tgt, f32).reshape(N, D))
    Wr = np.asarray(Wr, f32)
    br = np.asarray(br, f32)
    W1 = np.asarray(W1, f32)
    b1 = np.asarray(b1, f32)
    W2 = np.asarray(W2, f32)
    b2 = np.asarray(b2, f32)
    gamma = np.asarray(gamma, f32)
    beta = np.asarray(beta, f32)

    i1, i2, w1, w2 = _route_host(x, Wr, br)
    iota = np.broadcast_to(np.arange(512, dtype=f32), (128, 512))
    cst2 = np.ascontiguousarray(np.concatenate([
        np.broadcast_to(gamma, (128, D)),
        np.broadcast_to(beta, (128, D)),
        iota,
    ], axis=1))

    xt = x.T  # [D, N] view

    # per (expert, owner) token lists, split into A (first C2A) / B (rest)
    all_idx = []
    all_w = []
    all_loc = []
    for e in range(NC):
        sel = (i1 == e) | (i2 == e)
        idx_e = np.nonzero(sel)[0]
        w_e = np.where(i1[idx_e] == e, w1[idx_e], w2[idx_e]).astype(f32)
        seg_idx = np.zeros(C3, np.int64)
        seg_w = np.zeros(C3, f32)
        seg_loc = np.full(C3, PAD_IDX, np.int64)
        for r in range(NC):
            m = (idx_e >= r * TSLICE) & (idx_e < (r + 1) * TSLICE)
            li = idx_e[m]
            wi = w_e[m]
            if li.size > C2:
                return None
            na = min(li.size, C2A)
            a0 = r * C2A
            seg_idx[a0:a0 + na] = li[:na]
            seg_w[a0:a0 + na] = wi[:na]
            seg_loc[a0:a0 + na] = li[:na] - r * TSLICE
            nb = li.size - na
            if nb > 0:
                b0 = SPLA + r * C2B
                seg_idx[b0:b0 + nb] = li[na:]
                seg_w[b0:b0 + nb] = wi[na:]
                seg_loc[b0:b0 + nb] = li[na:] - r * TSLICE
        all_idx.append(seg_idx)
        all_w.append(seg_w)
        all_loc.append(seg_loc)

    in_maps = []
    for e in range(NC):
        seg_idx, seg_w = all_idx[e], all_w[e]
        xg = xt[:, seg_idx].reshape(ND, 128, C3)
        parts = []
        c0 = 0
        for tb in GBLOCKS3:
            parts.append(xg[:, :, c0:c0 + tb].transpose(1, 0, 2)
                         .reshape(128, ND * tb))
            c0 += tb
        xgsw = np.ascontiguousarray(np.concatenate(parts, axis=1)).astype(BF16)
        w1sw = np.ascontiguousarray(
            W1[e].T.reshape(ND, 128, F).transpose(1, 0, 2).reshape(128, ND * F)
        ).astype(BF16)
        w2sw = np.ascontiguousarray(
            W2[e].T.reshape(NF, 128, D).transpose(1, 0, 2).reshape(128, NF * D)
        ).astype(BF16)
        cst1 = np.ascontiguousarray(np.concatenate([
            np.broadcast_to(b2[e], (128, D)),
            b1[e].reshape(NF, 128).T,
            seg_w.reshape(NTG3, 128).T,
        ], axis=1))
        # locs for the RECEIVER side: core e receives segment e from every
        # expert core s: rows [s*C2, (s+1)*C2) of its a2a_out = expert s's
        # tokens owned by core e -> local idx = all_loc[s][e*C2 + k]
        loc_rows = np.full(C3, PAD_IDX, np.int64)
        for s in range(NC):
            loc_rows[s * C2A:(s + 1) * C2A] = \
                all_loc[s][e * C2A:(e + 1) * C2A]
            loc_rows[SPLA + s * C2B:SPLA + (s + 1) * C2B] = \
                all_loc[s][SPLA + e * C2B:SPLA + (e + 1) * C2B]
        locs = np.ascontiguousarray(
            loc_rows.astype(np.int32).reshape(NTG3, 128).T)
        in_maps.append({
            "xgsw": xgsw,
            "w1sw": w1sw,
            "w2sw": w2sw,
            "cst1": cst1,
            "cst2": cst2,
            "locs": locs,
            "xres": np.ascontiguousarray(x[e * TSLICE:(e + 1) * TSLICE]),
        })
    return in_maps


def _get_nc(kind="dense"):
    key = f"nc_{kind}"
    if key not in _CACHE:
        builders = {"dense": _build_nc, "sparse": _build_nc_sparse,
                    "sparse3": _build_nc_sparse3, "sparse4": _build_nc_sparse4}
        _CACHE[key] = builders[kind]()
    return _CACHE[key]


def _route_host(x, Wr, br):
    """fp32 router: returns (i1, i2, w1, w2) top-2 indices and renormalized
    softmax weights per token."""
    logits = x @ Wr.T.astype(np.float32) + br.astype(np.float32)  # [N, E]
    i1 = np.argmax(logits, axis=1)
    v1 = logits[np.arange(N), i1]
    masked = logits.copy()
    masked[np.arange(N), i1] = -np.inf
    i2 = np.argmax(masked, axis=1)
    v2 = masked[np.arange(N), i2]
    ew = np.exp(v2 - v1)            # <= 1
    w1 = 1.0 / (1.0 + ew)
    w2 = ew / (1.0 + ew)
    return i1, i2, w1.astype(np.float32), w2.astype(np.float32)


def _build_in_maps_sparse(tgt, Wr, br, W1, b1, W2, b2, gamma, beta):
    """Returns None if any expert's token count exceeds CCAP (caller falls
    back to the dense kernel)."""
    f32 = np.float32
    x = np.ascontiguousarray(np.asarray(tgt, f32).reshape(N, D))
    Wr = np.asarray(Wr, f32)
    br = np.asarray(br, f32)
    W1 = np.asarray(W1, f32)
    b1 = np.asarray(b1, f32)
    W2 = np.asarray(W2, f32)
    b2 = np.asarray(b2, f32)
    gamma = np.asarray(gamma, f32)
    beta = np.asarray(beta, f32)

    i1, i2, w1, w2 = _route_host(x, Wr, br)
    cst2 = np.ascontiguousarray(np.concatenate([
        np.broadcast_to(gamma, (128, D)),
        np.broadcast_to(beta, (128, D)),
    ], axis=1))

    xt = x.T  # [D, N] view

    in_maps = []
    for e in range(NC):
        sel1 = i1 == e
        sel2 = i2 == e
        idx = np.nonzero(sel1 | sel2)[0]
        if idx.size > CCAP:
            return None
        w = np.where(sel1[idx], w1[idx], w2[idx]).astype(f32)
        idx_pad = np.zeros(CCAP, np.int64)
        idx_pad[:idx.size] = idx
        scat_ids = np.full(CCAP, PAD_IDX, np.int32)
        scat_ids[:idx.size] = idx.astype(np.int32)
        wg = np.zeros(CCAP, f32)
        wg[:idx.size] = w

        # xgsw [128, ND*CCAP]: block-major columns, then d-tile, then token
        xg = xt[:, idx_pad].reshape(ND, 128, CCAP)   # [d0, p, c]
        parts = []
        c0 = 0
        for tb in GBLOCKS:
            parts.append(xg[:, :, c0:c0 + tb].transpose(1, 0, 2).reshape(128, ND * tb))
            c0 += tb
        xgsw = np.ascontiguousarray(np.concatenate(parts, axis=1)).astype(BF16)

        w1sw = np.ascontiguousarray(
            W1[e].T.reshape(ND, 128, F).transpose(1, 0, 2).reshape(128, ND * F)
        ).astype(BF16)
        w2sw = np.ascontiguousarray(
            W2[e].T.reshape(NF, 128, D).transpose(1, 0, 2).reshape(128, NF * D)
        ).astype(BF16)
        cst1 = np.ascontiguousarray(np.concatenate([
            np.broadcast_to(b2[e], (128, D)),
            b1[e].reshape(NF, 128).T,
            wg.reshape(NTG, 128).T,
        ], axis=1))
        in_maps.append({
            "xgsw": xgsw,
            "w1sw": w1sw,
            "w2sw": w2sw,
            "cst1": cst1,
            "cst2": cst2,
            "scat": np.ascontiguousarray(scat_ids.reshape(NTG, 128).T),
            "xres": np.ascontiguousarray(x[e * TSLICE:(e + 1) * TSLICE]),
        })
    return in_maps


def _build_in_maps(tgt, Wr, br, W1, b1, W2, b2, gamma, beta):
    f32 = np.float32
    tgt = np.asarray(tgt, dtype=f32)
    Wr = np.asarray(Wr, dtype=f32)
    br = np.asarray(br, dtype=f32)
    W1 = np.asarray(W1, dtype=f32)
    b1 = np.asarray(b1, dtype=f32)
    W2 = np.asarray(W2, dtype=f32)
    b2 = np.asarray(b2, dtype=f32)
    gamma = np.asarray(gamma, dtype=f32)
    beta = np.asarray(beta, dtype=f32)

    x = np.ascontiguousarray(tgt.reshape(N, D))
    xt = np.ascontiguousarray(x.T)                      # [D, N] fp32
    xtb = xt.astype(BF16)
    gmr = np.ascontiguousarray(np.broadcast_to(gamma, (128, D)))
    btr = np.ascontiguousarray(np.broadcast_to(beta, (128, D)))

    in_maps = []
    for e in range(NC):
        perm = [e] + [i for i in range(E) if i != e]
        in_maps.append({
            "xtf": xt,
            "xtb": xtb,
            "w1t": np.ascontiguousarray(W1[e].T).astype(BF16),   # [D, F]
            "w2t": np.ascontiguousarray(W2[e].T).astype(BF16),   # [F, D]
            "b1c": np.ascontiguousarray(b1[e].reshape(NF, 128).T),  # [128, NF]
            "b2r": np.ascontiguousarray(np.broadcast_to(b2[e], (128, D))),
            "wrt": np.ascontiguousarray(Wr[perm].T),             # [D, E]
            "brr": np.ascontiguousarray(np.broadcast_to(br[perm], (128, E))),
            "xres": np.ascontiguousarray(x[e * TSLICE:(e + 1) * TSLICE]),
            "gmr": gmr,
            "btr": btr,
        })
    return in_maps


def _assemble(results):
    out = np.concatenate([results[r]["out"] for r in range(NC)], axis=0)
    return np.ascontiguousarray(out.reshape(B, S, D).astype(np.float32))


def _get_runner(kind="dense"):
    """Build (once per kind) a cached jitted SPMD executor mirroring
    concourse.bass2jax.run_bass_via_pjrt, so repeat kernel() calls skip
    recompilation."""
    rkey = f"runner_{kind}"
    if rkey in _CACHE:
        return _CACHE[rkey]

    import jax
    from jax.sharding import Mesh, PartitionSpec
    from jax.experimental.shard_map import shard_map
    import concourse.mybir as mybir
    from concourse import bass2jax

    nc = _get_nc(kind)
    bass2jax.install_neuronx_cc_hook()

    partition_name = nc.partition_id_tensor.name if nc.partition_id_tensor else None
    in_names, out_names, out_avals, zero_outs = [], [], [], []
    for alloc in nc.m.functions[0].allocations:
        if not isinstance(alloc, mybir.MemoryLocationSet):
            continue
        name = alloc.memorylocations[0].name
        if alloc.kind == "ExternalInput":
            if name != partition_name:
                in_names.append(name)
        elif alloc.kind == "ExternalOutput":
            shape = tuple(alloc.tensor_shape)
            dtype = mybir.dt.np(alloc.dtype)
            out_names.append(name)
            out_avals.append(jax.core.ShapedArray(shape, dtype))
            zero_outs.append(np.zeros(shape, dtype))
    n_params = len(in_names)
    n_outs = len(out_avals)
    all_in_names = list(in_names) + list(out_names)
    if partition_name is not None:
        all_in_names.append(partition_name)

    donate = tuple(range(n_params, n_params + n_outs))

    def _body(*args):
        operands = list(args)
        if partition_name is not None:
            operands.append(bass2jax.partition_id_tensor())
        outs = bass2jax._bass_exec_p.bind(
            *operands,
            out_avals=tuple(out_avals),
            in_names=tuple(all_in_names),
            out_names=tuple(out_names),
            lowering_input_output_aliases=(),
            sim_require_finite=True,
            sim_require_nnan=True,
            nc=nc,
        )
        return tuple(outs)

    devices = jax.devices()[:NC]
    mesh = Mesh(np.asarray(devices), ("core",))
    in_specs = (PartitionSpec("core"),) * (n_params + n_outs)
    out_specs = (PartitionSpec("core"),) * n_outs
    sharded = jax.jit(
        shard_map(
            _body, mesh=mesh, in_specs=in_specs, out_specs=out_specs,
            check_rep=False,
        ),
        donate_argnums=donate,
        keep_unused=True,
    )

    def run(in_maps):
        per_core = [[np.asarray(m[name]) for name in in_names] for m in in_maps]
        concat_in = [
            np.concatenate([per_core[c][i] for c in range(NC)], axis=0)
            for i in range(n_params)
        ]
        concat_zeros = [
            np.zeros((NC * z.shape[0], *z.shape[1:]), z.dtype) for z in zero_outs
        ]
        out_arrs = sharded(*concat_in, *concat_zeros)
        return [
            {
                name: np.asarray(out_arrs[i]).reshape(NC, *out_avals[i].shape)[c]
                for i, name in enumerate(out_names)
            }
            for c in range(NC)
        ]

    _CACHE[rkey] = run
    return run


def kernel(tgt, Wr, br, W1, b1, W2, b2, gamma, beta):
    args = (tgt, Wr, br, W1, b1, W2, b2, gamma, beta)
    # fastest path: balanced ownership, capacity C4
    try:
        built = _build_in_maps_sparse4(*args)
    except Exception:
        built = None
    if built is not None:
        in_maps4, tok_lists = built
        try:
            results = _get_runner("sparse4")(in_maps4)
            cat = np.concatenate([results[r]["out"] for r in range(NC)], axis=0)
            full = np.empty((N, D), np.float32)
            full[np.concatenate(tok_lists)] = cat
            return np.ascontiguousarray(full.reshape(B, S, D))
        except Exception:
            pass
    # next: per-(expert, owner)-segment capacity C2, identity ownership
    try:
        in_maps = _build_in_maps_sparse3(*args)
    except Exception:
        in_maps = None
    if in_maps is not None:
        try:
            results = _get_runner("sparse3")(in_maps)
            return _assemble(results)
        except Exception:
            pass
    # second chance: per-expert capacity CCAP (tolerates skewed owners)
    try:
        in_maps = _build_in_maps_sparse(*args)
    except Exception:
        in_maps = None
    if in_maps is not None:
        try:
            results = _get_runner("sparse")(in_maps)
            return _assemble(results)
        except Exception:
            pass
    # Dense fallback: capacity overflow or any sparse-path failure.
    in_maps = _build_in_maps(*args)
    try:
        results = _get_runner("dense")(in_maps)
    except Exception:
        # Last resort: the well-tested (but recompiling) library path.
        from concourse.bass_utils import run_bass_kernel_spmd
        res = run_bass_kernel_spmd(
            _get_nc("dense"), in_maps, core_ids=list(range(NC))
        )
        results = res.results
    return _assemble(results)

